# revision 35
# baseline (speedup 1.0000x reference)
"""GQA attention (B=2, L=2048, HID=2048, 32 Q heads / 8 KV heads) on 8 TRN2 cores.

Sharding: data-parallel on batch (2) x tensor-parallel on heads (4).
Core c: batch b = c//4, TP rank r = c%4 owns q heads {8r..8r+7} (whole GQA
groups: kv heads 2r, 2r+1). Compute in bf16 on the TensorEngine (fp32 PSUM
accumulation), fp32 softmax statistics. Per-core pipeline:
  1. kT = Wk_c.T @ kv_b.T and vT = Wv_c.T @ kv_b.T streamed over 16 row
     chunks of kvT; vT transposed back to [pos, dim] tiles via TensorE.
  2. QT = (Wq_c * scale).T @ query_b.T -> [512, L] bf16 (pair-major rows).
  3. per head-pair (g0-head, g1-head) and q-chunk j: scores^T = kT.T @ QT
     (two row-packed K=64 matmuls), exp on ACT, multiplicative mask
     (host-exp'd band tiles) on DVE, PV^T with ones-column giving the
     softmax denominator in PSUM row 64. PV results are copied out of PSUM
     immediately on DVE (releasing banks); normalization (reciprocal +
     gpsimd partition broadcast + mul) happens off the critical path.
  4. AllGather attnT (bf16) per (q-chunk, pair-group) over the 4-rank TP
     group.  Collectives are the ONLY thing on the gpsimd queue besides
     broadcasts, so their serialization does not stall the attention pipe.
  5. out_c[:, 512r:+512] = attnT_full.T @ Wo_perm_c + bo_c, split into two
     8-matmul half-accumulations per 128-row tile, interleaved into the
     attention instruction stream with enough slack to cover AG latency.
Host assembles [2, 2048, 2048] from per-core [2048, 512] f32 slabs.

Mask handling is input-driven: the effective additive mask (attn_mask +
key-padding) is classified on host per (q-chunk, k-tile) block as
all-masked (skip), all-zero (no op), or band (exp(mask) shipped and
multiplied into exp(scores)).
"""

import numpy as np
import ml_dtypes
import concourse.bass as bass
import concourse.mybir as mybir
import concourse.tile as tile
from concourse import bacc
from concourse.bass_utils import run_bass_kernel_spmd

F32 = mybir.dt.float32
BF16 = mybir.dt.bfloat16
AF = mybir.ActivationFunctionType
NPBF16 = ml_dtypes.bfloat16

B, L, HID = 2, 2048, 2048
NH, D, NKV = 32, 64, 8
SCALE = 0.125
N_CORES = 8
TPR = 4          # TP ranks per batch group
NPAIR = 4        # head pairs per core (g0-head, g1-head)
LQC = 512        # Lq chunk for attention (PSUM-bank sized)
NJ = L // LQC    # 4
KT = 128         # k-position tile
NI = L // KT     # 16
NEG_THRESH = -1.0e8
NCHUNK = HID // 128  # 16

_graph_cache = {}
last_results = None  # BassKernelResults of the most recent run (for test harness)


def _classify_blocks(eff_masks):
    """eff_masks: list of B arrays [L, L] (q, k). Returns (live, band_list)
    where live[j] is the ascending list of k-tiles to compute for q-chunk j and
    band_list orders the (j, i) blocks that need explicit mask values."""
    live = {}
    band_list = []
    for j in range(NJ):
        lv = []
        for i in range(NI):
            subs = [m[j * LQC:(j + 1) * LQC, i * KT:(i + 1) * KT] for m in eff_masks]
            if all((s <= NEG_THRESH).all() for s in subs):
                continue  # fully masked in every batch: contributes exactly 0
            lv.append(i)
            if not all((s == 0.0).all() for s in subs):
                band_list.append((j, i))
        live[j] = lv
    return live, band_list


def _build_graph(live_key, band_key):
    key = (live_key, band_key)
    if key in _graph_cache:
        return _graph_cache[key]

    live = {j: list(lv) for j, lv in live_key}
    band_list = list(band_key)
    band_idx = {ji: n for n, ji in enumerate(band_list)}
    nband = max(1, len(band_list))

    nc = bacc.Bacc("TRN2", target_bir_lowering=False, debug=False,
                   num_devices=N_CORES)

    qT = nc.dram_tensor("qT", [HID, L], BF16, kind="ExternalInput")
    kvT = nc.dram_tensor("kvT", [HID, L], BF16, kind="ExternalInput")
    wq = nc.dram_tensor("wq", [128, 16 * 512], BF16, kind="ExternalInput")
    bq = nc.dram_tensor("bq", [128, 4], F32, kind="ExternalInput")
    wk = nc.dram_tensor("wk", [128, 2048], BF16, kind="ExternalInput")
    bk = nc.dram_tensor("bk", [128, 1], F32, kind="ExternalInput")
    wv = nc.dram_tensor("wv", [128, 2048], BF16, kind="ExternalInput")
    bv = nc.dram_tensor("bv", [128, 1], F32, kind="ExternalInput")
    wo = nc.dram_tensor("wo", [128, 16 * 512], BF16, kind="ExternalInput")
    bo = nc.dram_tensor("bo", [1, 512], BF16, kind="ExternalInput")
    band = nc.dram_tensor("band", [128, nband * 512], BF16, kind="ExternalInput")
    ones = nc.dram_tensor("ones", [128, 128], BF16, kind="ExternalInput")
    ident = nc.dram_tensor("ident", [128, 128], BF16, kind="ExternalInput")
    out_ext = nc.dram_tensor("out", [L, 512], F32, kind="ExternalOutput")

    ag_warm_in = nc.dram_tensor("ag_warm_in", [256, 512], BF16)
    ag_warm_out = nc.dram_tensor("ag_warm_out", [1024, 512], BF16)
    ag_in = [[nc.dram_tensor(f"ag_in{j}_{g}", [256, 512], BF16)
              for g in range(2)] for j in range(NJ)]
    ag_out = [[nc.dram_tensor(f"ag_out{j}_{g}", [1024, 512], BF16)
               for g in range(2)] for j in range(NJ)]
    groups = [[0, 1, 2, 3], [4, 5, 6, 7]]

    with tile.TileContext(nc) as tc:
        with tc.tile_pool(name="persist", bufs=1) as persist:
            ones_sb = persist.tile([128, 128], BF16, tag="ones")
            nc.sync.dma_start(ones_sb[:], ones[:])
            # weights needed first: wk/wv lead the scalar queue
            wk_sb = persist.tile([128, 2048], BF16, tag="wk")
            nc.scalar.dma_start(wk_sb[:], wk[:])
            wv_sb = persist.tile([128, 2048], BF16, tag="wv")
            nc.scalar.dma_start(wv_sb[:], wv[:])
            # warm up the collective path with the REAL transfer shape so the
            # first in-loop AllGather doesn't pay the plan-setup cost
            nc.gpsimd.dma_start(ag_warm_in[0:128, 0:128], ones[:])
            nc.gpsimd.dma_start(ag_warm_in[128:256, 0:128], ones[:])
            nc.gpsimd.collective_compute(
                "AllGather", mybir.AluOpType.bypass,
                replica_groups=groups,
                ins=[ag_warm_in[:]], outs=[ag_warm_out[:]])

            ident_sb = persist.tile([128, 128], BF16, tag="ident")
            nc.scalar.dma_start(ident_sb[:], ident[:])
            bq_sb = persist.tile([128, 4], F32, tag="bq")
            nc.scalar.dma_start(bq_sb[:], bq[:])
            bk_sb = persist.tile([128, 1], F32, tag="bk")
            nc.scalar.dma_start(bk_sb[:], bk[:])
            bv_sb = persist.tile([128, 1], F32, tag="bv")
            nc.scalar.dma_start(bv_sb[:], bv[:])
            bo_sb = persist.tile([1, 512], BF16, tag="bo")
            nc.scalar.dma_start(bo_sb[:], bo[:])
            wq_sb = persist.tile([128, 16 * 512], BF16, tag="wq")
            nc.scalar.dma_start(wq_sb[:], wq[:])
            wo_sb = persist.tile([128, 16 * 512], BF16, tag="wo")
            band_sb = persist.tile([128, nband * 512], BF16, tag="band")
            # attention-era weights go over the otherwise-idle gpsimd
            # (SWDGE) queue so they land well before attention starts and
            # never contend with the kvT/qT input streams
            nc.gpsimd.dma_start(band_sb[:], band[:])
            nc.gpsimd.dma_start(wo_sb[:], wo[:])

            kT_sb = persist.tile([128, L], BF16, tag="kT")
            v_sb = [persist.tile([128, 130], BF16, tag=f"v{t}", name=f"v{t}")
                    for t in range(NI)]
            for t in range(NI):
                nc.vector.memset(v_sb[t][:, 64:65], 1.0)
                nc.vector.memset(v_sb[t][:, 129:130], 1.0)
            QT_sb = [persist.tile([128, L], BF16, tag=f"qt{m}", name=f"qt{m}")
                     for m in range(NPAIR)]
            # gathered attnT blocks, filled per (j, g2) as AllGathers complete
            ag_sb = [persist.tile([128, L], BF16, tag=f"ag{kk}", name=f"ag{kk}")
                     for kk in range(16)]

            # ---- KV projection: kT and vT streamed over 16 row chunks
            with tc.tile_pool(name="kv_era", bufs=1) as kv_era:
                vT_sb = kv_era.tile([128, L], BF16, tag="vT")
                with (
                    tc.tile_pool(name="kv_stream", bufs=4) as kv_stream,
                    tc.tile_pool(name="kv_psum", bufs=1, space="PSUM") as kv_psum,
                ):
                    psk = kv_psum.tile([128, L], F32, tag="psk")
                    psv = kv_psum.tile([128, L], F32, tag="psv")
                    for k in range(NCHUNK):
                        ch = kv_stream.tile([128, L], BF16, tag="kvch")
                        eng = nc.sync if k % 2 == 0 else nc.scalar
                        # two half-row DMAs so matmuls start on the first
                        # half while the second is still in flight
                        eng.dma_start(ch[:, 0:1024],
                                      kvT[128 * k:128 * (k + 1), 0:1024])
                        eng.dma_start(ch[:, 1024:2048],
                                      kvT[128 * k:128 * (k + 1), 1024:2048])
                        for jj in range(4):
                            nc.tensor.matmul(
                                psk[:, 512 * jj:512 * (jj + 1)],
                                wk_sb[:, 128 * k:128 * (k + 1)],
                                ch[:, 512 * jj:512 * (jj + 1)],
                                start=(k == 0), stop=(k == NCHUNK - 1),
                                skip_group_check=True)
                        for jj in range(4):
                            nc.tensor.matmul(
                                psv[:, 512 * jj:512 * (jj + 1)],
                                wv_sb[:, 128 * k:128 * (k + 1)],
                                ch[:, 512 * jj:512 * (jj + 1)],
                                start=(k == 0), stop=(k == NCHUNK - 1),
                                skip_group_check=True)
                    for jj in range(4):
                        nc.scalar.activation(
                            kT_sb[:, 512 * jj:512 * (jj + 1)],
                            psk[:, 512 * jj:512 * (jj + 1)],
                            AF.Identity, bias=bk_sb[:])
                        nc.scalar.activation(
                            vT_sb[:, 512 * jj:512 * (jj + 1)],
                            psv[:, 512 * jj:512 * (jj + 1)],
                            AF.Identity, bias=bv_sb[:])

                # transpose vT -> v tiles [pos, dim] (TensorE, exact)
                with tc.tile_pool(name="tr_psum", bufs=2, space="PSUM") as trp:
                    for t in range(NI):
                        tp = trp.tile([128, 128], BF16, tag="tp")
                        nc.tensor.transpose(
                            tp[:], vT_sb[:, 128 * t:128 * (t + 1)], ident_sb[:])
                        nc.vector.tensor_copy(v_sb[t][:, 0:64], tp[:, 0:64])
                        nc.vector.tensor_copy(v_sb[t][:, 65:129], tp[:, 64:128])

            # ---- Q projection (two 1024-col halves, 8 PSUM banks each)
            with (
                tc.tile_pool(name="q_stream", bufs=4) as q_stream,
                tc.tile_pool(name="q_psum", bufs=1, space="PSUM") as q_psum,
            ):
                for jp in range(2):
                    psq = [q_psum.tile([128, 1024], F32, tag=f"psq{m}",
                                       name=f"psq{m}") for m in range(4)]
                    for k in range(NCHUNK):
                        ch = q_stream.tile([128, 1024], BF16, tag="qch")
                        eng = nc.sync if k % 2 == 0 else nc.scalar
                        eng.dma_start(
                            ch[:], qT[128 * k:128 * (k + 1),
                                      1024 * jp:1024 * (jp + 1)])
                        for m in range(4):
                            for jj in range(2):
                                nc.tensor.matmul(
                                    psq[m][:, 512 * jj:512 * (jj + 1)],
                                    wq_sb[:, 512 * k + 128 * m:
                                          512 * k + 128 * (m + 1)],
                                    ch[:, 512 * jj:512 * (jj + 1)],
                                    start=(k == 0), stop=(k == NCHUNK - 1),
                                    skip_group_check=True)
                    for m in range(4):
                        # split the final writeback between ACT and DVE so
                        # the PSUM pool hands off to the attention pools
                        # ~2x sooner after the last matmul
                        if jp == 1 and m >= 2:
                            nc.vector.tensor_scalar_add(
                                QT_sb[m][:, 1024 * jp:1024 * (jp + 1)],
                                psq[m][:], bq_sb[:, m:m + 1])
                        else:
                            for jj in range(2):
                                j = 2 * jp + jj
                                nc.scalar.activation(
                                    QT_sb[m][:, 512 * j:512 * (j + 1)],
                                    psq[m][:, 512 * jj:512 * (jj + 1)],
                                    AF.Identity, bias=bq_sb[:, m:m + 1])



            # ---- Attention (j-outer, pairs in order [2,3,0,1]) with the
            # ---- output projection interleaved as half-accumulations
            with (
                tc.tile_pool(name="pt_pool", bufs=3) as pt_pool,
                tc.tile_pool(name="at_pool", bufs=3) as at_pool,
                tc.tile_pool(name="rc_pool", bufs=2) as rc_pool,
                tc.tile_pool(name="o_out", bufs=2) as o_out,
                tc.tile_pool(name="qk_psum", bufs=2, space="PSUM") as qk_psum,
                tc.tile_pool(name="pv_psum", bufs=1, space="PSUM") as pv_psum,
                tc.tile_pool(name="o_psum", bufs=2, space="PSUM") as o_psum,
            ):
                pending_scatters = []

                def _emit_scatter(j_, g2_):
                    for rp in range(TPR):
                        for q01 in range(2):
                            kk = 8 * g2_ + 2 * rp + q01
                            nc.sync.dma_start(
                                ag_sb[kk][:, 512 * j_:512 * (j_ + 1)],
                                ag_out[j_][g2_][
                                    256 * rp + 128 * q01:
                                    256 * rp + 128 * (q01 + 1), :])

                def flush_scatters(now_blk):
                    # emit scatter DMAs only once the AllGather has had ~2
                    # pair-blocks to complete, so they never head-of-line
                    # block the sync queue for the ag_in writes behind them
                    while pending_scatters and (
                            now_blk is None
                            or pending_scatters[0][2] + 2 <= now_blk):
                        j_, g2_, _ = pending_scatters.pop(0)
                        _emit_scatter(j_, g2_)

                def ensure_scatter(j_, g2_):
                    for e in list(pending_scatters):
                        if e[0] == j_ and e[1] == g2_:
                            pending_scatters.remove(e)
                            _emit_scatter(j_, g2_)

                def attn_block(j, pr, mid_cb=None):
                    lv = live[j]
                    pva = pv_psum.tile([65, 512], F32, tag="pva")
                    pvb = pv_psum.tile([65, 512], F32, tag="pvb")
                    for n, i in enumerate(lv):
                        if n == 2 and mid_cb is not None:
                            mid_cb()
                        ps = qk_psum.tile([128, 1024], F32, tag="qk")
                        nc.tensor.matmul(
                            ps[:, 0:512],
                            kT_sb[0:64, 128 * i:128 * (i + 1)],
                            QT_sb[pr][0:64, 512 * j:512 * (j + 1)],
                            start=True, stop=True, skip_group_check=True)
                        nc.tensor.matmul(
                            ps[:, 512:1024],
                            kT_sb[64:128, 128 * i:128 * (i + 1)],
                            QT_sb[pr][64:128, 512 * j:512 * (j + 1)],
                            start=True, stop=True, skip_group_check=True)
                        pt = pt_pool.tile([128, 1024], BF16, tag="pt")
                        nc.scalar.activation(pt[:], ps[:], AF.Exp)
                        if (j, i) in band_idx:
                            nb = band_idx[(j, i)]
                            bt = band_sb[:, 512 * nb:512 * (nb + 1)]
                            nc.vector.tensor_mul(pt[:, 0:512], pt[:, 0:512], bt)
                            nc.vector.tensor_mul(pt[:, 512:1024],
                                                 pt[:, 512:1024], bt)
                        nc.tensor.matmul(
                            pva[:], v_sb[i][:, 0:65], pt[:, 0:512],
                            start=(n == 0), stop=(n == len(lv) - 1),
                            skip_group_check=True)
                        nc.tensor.matmul(
                            pvb[:], v_sb[i][:, 65:130], pt[:, 512:1024],
                            start=(n == 0), stop=(n == len(lv) - 1),
                            skip_group_check=True)
                    # copy PV results out of PSUM promptly (DVE) so the pv
                    # banks free up for the next pair regardless of gpsimd
                    au = at_pool.tile([65, 1024], BF16, tag="au")
                    nc.vector.tensor_copy(au[:, 0:512], pva[:])
                    nc.vector.tensor_copy(au[:, 512:1024], pvb[:])
                    # normalization: 1/denominator, broadcast, multiply.
                    # Keep everything bf16 so the DVE muls hit the 2x mode.
                    rsa = rc_pool.tile([1, 512], F32, tag="rsa")
                    rsb = rc_pool.tile([1, 512], F32, tag="rsb")
                    nc.vector.tensor_copy(rsa[:], au[64:65, 0:512])
                    nc.vector.tensor_copy(rsb[:], au[64:65, 512:1024])
                    rra = rc_pool.tile([1, 512], F32, tag="rra")
                    rrb = rc_pool.tile([1, 512], F32, tag="rrb")
                    nc.vector.reciprocal_approx_fast(out=rra[:], in_=rsa[:])
                    nc.vector.reciprocal_approx_fast(out=rrb[:], in_=rsb[:])
                    rh = rc_pool.tile([1, 1024], BF16, tag="rh")
                    nc.vector.tensor_copy(rh[:, 0:512], rra[:])
                    nc.vector.tensor_copy(rh[:, 512:1024], rrb[:])
                    rb = rc_pool.tile([64, 1024], BF16, tag="rb")
                    nc.gpsimd.partition_broadcast(rb[:, 0:512], rh[:, 0:512])
                    nc.gpsimd.partition_broadcast(rb[:, 512:1024],
                                                  rh[:, 512:1024])
                    ata = at_pool.tile([64, 1024], BF16, tag="ata")
                    nc.vector.tensor_mul(ata[:, 0:512], au[0:64, 0:512],
                                         rb[:, 0:512])
                    nc.vector.tensor_mul(ata[:, 512:1024],
                                         au[0:64, 512:1024], rb[:, 512:1024])
                    g2, p01 = pr // 2, pr % 2
                    nc.sync.dma_start(
                        ag_in[j][g2][128 * p01:128 * p01 + 64, :],
                        ata[:, 0:512])
                    nc.sync.dma_start(
                        ag_in[j][g2][128 * p01 + 64:128 * (p01 + 1), :],
                        ata[:, 512:1024])
                    if p01 == 1:
                        nc.gpsimd.collective_compute(
                            "AllGather", mybir.AluOpType.bypass,
                            replica_groups=groups,
                            ins=[ag_in[j][g2][:]], outs=[ag_out[j][g2][:]])
                        pending_scatters.append((j, g2, cur_blk[0]))

                pso_map = {}

                def oproj_half(t, half):
                    # half 0: bias + gathered g2=1 blocks (kk 8..15);
                    # half 1: g2=0 blocks (kk 0..7), then writeback.
                    ensure_scatter(t // 4, 1 if half == 0 else 0)
                    if half == 0:
                        pso = o_psum.tile([128, 512], F32, tag="pso")
                        pso_map[t] = pso
                        nc.tensor.matmul(pso[:], ones_sb[0:1, :], bo_sb[:],
                                         start=True, stop=False,
                                         skip_group_check=True)
                        for kk in range(8, 16):
                            nc.tensor.matmul(
                                pso[:], ag_sb[kk][:, 128 * t:128 * (t + 1)],
                                wo_sb[:, 512 * kk:512 * (kk + 1)],
                                start=False, stop=False, skip_group_check=True)
                    else:
                        pso = pso_map.pop(t)
                        for kk in range(8):
                            nc.tensor.matmul(
                                pso[:], ag_sb[kk][:, 128 * t:128 * (t + 1)],
                                wo_sb[:, 512 * kk:512 * (kk + 1)],
                                start=False, stop=(kk == 7),
                                skip_group_check=True)
                        osb = o_out.tile([128, 512], F32, tag="osb")
                        nc.vector.tensor_copy(osb[:], pso[:])
                        nc.sync.dma_start(out_ext[128 * t:128 * (t + 1), :],
                                          osb[:])

                PR_ORDER = [2, 3, 0, 1]
                oq = []   # entries: [t, next_half, ready0, ready1]
                cur_blk = [0]

                def pop_ready(blk_, budget):
                    # drain up to `budget` half-units.  Entries may be taken
                    # out of FIFO order (a ready h0 can jump a cc-gated h1),
                    # but at most 2 tiles may hold an o_psum slot (bufs=2).
                    while budget > 0:
                        n_open = sum(1 for e in oq if e[1] == 1)
                        done = False
                        for e in oq:
                            rdy = e[2] if e[1] == 0 else e[3]
                            if blk_ is not None and (rdy is None or rdy > blk_):
                                continue
                            if e[1] == 0 and n_open >= 2:
                                continue
                            oproj_half(e[0], e[1])
                            if e[1] == 0:
                                e[1] = 1
                            else:
                                oq.remove(e)
                            budget -= 1
                            done = True
                            break
                        if not done:
                            break
                for j in range(NJ):
                    for pr in PR_ORDER:
                        flush_scatters(cur_blk[0])
                        # drain one oproj half mid-block and one at the end:
                        # spreading the bursts keeps the exp stream fed
                        attn_block(j, pr,
                                   mid_cb=lambda: pop_ready(cur_blk[0], 1))
                        cur_blk[0] += 1
                        blk = cur_blk[0]
                        if pr == 3:   # g2=1 AllGather just issued
                            slack = 2 if j == NJ - 1 else 4
                            for t in range(4 * j, 4 * j + 4):
                                oq.append([t, 0, blk + slack, None])
                        if pr == 1:   # g2=0 AllGather issued (chunk done)
                            for e in oq:
                                if 4 * j <= e[0] < 4 * j + 4:
                                    e[3] = blk + 4
                        pop_ready(blk, 1)
                flush_scatters(None)
                pop_ready(None, len(oq) * 2 + 4)

    nc.compile()
    _graph_cache[key] = nc
    return nc


def _pack16(a):
    """[2048, X] -> [128, 16*X] with row-chunk k at cols [X*k, X*(k+1))."""
    x = a.shape[1]
    return np.ascontiguousarray(
        a.reshape(16, 128, x).transpose(1, 0, 2).reshape(128, 16 * x))


def kernel(query, kv, Wq, bq, Wkv, bkv, Wo, bo, attn_mask, key_padding_mask):
    global last_results
    query = np.asarray(query, np.float32)
    kv = np.asarray(kv, np.float32)
    Wq = np.asarray(Wq, np.float32)
    bq = np.asarray(bq, np.float32)
    Wkv = np.asarray(Wkv, np.float32)
    bkv = np.asarray(bkv, np.float32)
    Wo = np.asarray(Wo, np.float32)
    bo = np.asarray(bo, np.float32)
    attn_mask = np.asarray(attn_mask, np.float32)
    kpm = np.asarray(key_padding_mask)

    eff = [attn_mask + np.where(kpm[b], np.float32(-1e9), np.float32(0.0))[None, :]
           for b in range(B)]
    live, band_list = _classify_blocks(eff)
    live_key = tuple((j, tuple(lv)) for j, lv in sorted(live.items()))
    band_key = tuple(band_list)

    nc = _build_graph(live_key, band_key)

    # Host-side shard prep (bf16 for all TensorEngine operands)
    qTh = [np.ascontiguousarray(query[b].T.astype(NPBF16)) for b in range(B)]
    kvTh = [np.ascontiguousarray(kv[b].T.astype(NPBF16)) for b in range(B)]
    bandh = []
    with np.errstate(over="ignore", under="ignore"):
        for b in range(B):
            if band_list:
                bt = np.stack(
                    [np.exp(eff[b][j * LQC:(j + 1) * LQC,
                                   i * KT:(i + 1) * KT].T)
                     for (j, i) in band_list]).astype(NPBF16)
                bandh.append(np.ascontiguousarray(
                    bt.transpose(1, 0, 2).reshape(KT, len(band_list) * LQC)))
            else:
                bandh.append(np.zeros((KT, 512), NPBF16))
    ones_h = np.ones((128, 128), NPBF16)
    ident_h = np.eye(128, dtype=NPBF16)

    Wq_h = Wq.reshape(HID, NH, D)
    bq_h = bq.reshape(NH, D)
    Wo_h = Wo.reshape(NH, D, HID)

    in_maps = []
    for c in range(N_CORES):
        b, r = c // TPR, c % TPR
        heads_q = [8 * r + pr + 4 * e for pr in range(NPAIR) for e in range(2)]
        perm_glob = [8 * rp + (2 * g2 + p01) + 4 * e
                     for g2 in range(2) for rp in range(TPR)
                     for p01 in range(2) for e in range(2)]
        wq_c = _pack16(
            (Wq_h[:, heads_q, :].reshape(HID, 512) * SCALE).astype(NPBF16))
        bq_c = np.ascontiguousarray(
            (bq_h[heads_q].reshape(512) * SCALE).reshape(4, 128).T)
        wk_c = _pack16(Wkv[:, 128 * r:128 * (r + 1)].astype(NPBF16))
        bk_c = np.ascontiguousarray(bkv[128 * r:128 * (r + 1)])[:, None]
        wv_c = _pack16(Wkv[:, 512 + 128 * r:512 + 128 * (r + 1)].astype(NPBF16))
        bv_c = np.ascontiguousarray(
            bkv[512 + 128 * r:512 + 128 * (r + 1)])[:, None]
        wo_c = _pack16(
            Wo_h[perm_glob].reshape(HID, HID)[:, 512 * r:512 * (r + 1)]
            .astype(NPBF16))
        bo_c = np.ascontiguousarray(
            bo[512 * r:512 * (r + 1)].astype(NPBF16))[None, :]
        in_maps.append({
            "qT": qTh[b], "kvT": kvTh[b],
            "wq": wq_c, "bq": bq_c,
            "wk": wk_c, "bk": bk_c,
            "wv": wv_c, "bv": bv_c,
            "wo": wo_c, "bo": bo_c,
            "band": bandh[b], "ones": ones_h, "ident": ident_h,
        })

    last_results = run_bass_kernel_spmd(nc, in_maps, core_ids=list(range(N_CORES)))

    out = np.empty((B, L, HID), np.float32)
    for c in range(N_CORES):
        b, r = c // TPR, c % TPR
        out[b, :, 512 * r:512 * (r + 1)] = last_results.results[c]["out"]
    return out


# revision 40
# speedup vs baseline: 1.1072x; 1.1072x over previous
"""GQA attention (B=2, L=2048, HID=2048, 32 Q heads / 8 KV heads) on 8 TRN2 cores.

Sharding: data-parallel on batch (2) x tensor-parallel on heads (4).
Core c: batch b = c//4, TP rank r = c%4 owns q heads {8r..8r+7} (whole GQA
groups: kv heads 2r, 2r+1). Compute in bf16 on the TensorEngine (fp32 PSUM
accumulation), fp32 softmax statistics. Per-core pipeline:
  1. kT = Wk_c.T @ kv_b.T and vT = Wv_c.T @ kv_b.T streamed over 16 row
     chunks of kvT; vT transposed back to [pos, dim] tiles via TensorE.
  2. QT = (Wq_c * scale).T @ query_b.T -> [512, L] bf16 (pair-major rows).
  3. per head-pair (g0-head, g1-head) and q-chunk j: scores^T = kT.T @ QT
     (two row-packed K=64 matmuls), exp on ACT, multiplicative mask
     (host-exp'd band tiles) on DVE, PV^T with ones-column giving the
     softmax denominator in PSUM row 64. PV results are copied out of PSUM
     immediately on DVE (releasing banks); normalization (reciprocal +
     gpsimd partition broadcast + mul) happens off the critical path.
  4. AllGather attnT (bf16) per (q-chunk, pair-group) over the 4-rank TP
     group.  Collectives are the ONLY thing on the gpsimd queue besides
     broadcasts, so their serialization does not stall the attention pipe.
  5. out_c[:, 512r:+512] = attnT_full.T @ Wo_perm_c + bo_c, split into two
     8-matmul half-accumulations per 128-row tile, interleaved into the
     attention instruction stream with enough slack to cover AG latency.
Host assembles [2, 2048, 2048] from per-core [2048, 512] f32 slabs.

Mask handling is input-driven: the effective additive mask (attn_mask +
key-padding) is classified on host per (q-chunk, k-tile) block as
all-masked (skip), all-zero (no op), or band (exp(mask) shipped and
multiplied into exp(scores)).
"""

import numpy as np
import ml_dtypes
import concourse.bass as bass
import concourse.mybir as mybir
import concourse.tile as tile
from concourse import bacc
from concourse.bass_utils import run_bass_kernel_spmd

F32 = mybir.dt.float32
BF16 = mybir.dt.bfloat16
AF = mybir.ActivationFunctionType
NPBF16 = ml_dtypes.bfloat16

B, L, HID = 2, 2048, 2048
NH, D, NKV = 32, 64, 8
SCALE = 0.125
N_CORES = 8
TPR = 4          # TP ranks per batch group
NPAIR = 4        # head pairs per core (g0-head, g1-head)
LQC = 512        # Lq chunk for attention (PSUM-bank sized)
NJ = L // LQC    # 4
KT = 128         # k-position tile
NI = L // KT     # 16
NEG_THRESH = -1.0e8
NCHUNK = HID // 128  # 16

_graph_cache = {}
last_results = None  # BassKernelResults of the most recent run (for test harness)


def _classify_blocks(eff_masks):
    """eff_masks: list of B arrays [L, L] (q, k). Returns (live, band_list)
    where live[j] is the ascending list of k-tiles to compute for q-chunk j and
    band_list orders the (j, i) blocks that need explicit mask values."""
    live = {}
    band_list = []
    for j in range(NJ):
        lv = []
        for i in range(NI):
            subs = [m[j * LQC:(j + 1) * LQC, i * KT:(i + 1) * KT] for m in eff_masks]
            if all((s <= NEG_THRESH).all() for s in subs):
                continue  # fully masked in every batch: contributes exactly 0
            lv.append(i)
            if not all((s == 0.0).all() for s in subs):
                band_list.append((j, i))
        live[j] = lv
    return live, band_list


def _build_graph(live_key, band_key):
    key = (live_key, band_key)
    if key in _graph_cache:
        return _graph_cache[key]

    live = {j: list(lv) for j, lv in live_key}
    band_list = list(band_key)
    band_idx = {ji: n for n, ji in enumerate(band_list)}
    nband = max(1, len(band_list))

    nc = bacc.Bacc("TRN2", target_bir_lowering=False, debug=False,
                   num_devices=N_CORES)

    qT = nc.dram_tensor("qT", [HID, L], BF16, kind="ExternalInput")
    kvT = nc.dram_tensor("kvT", [HID, L], BF16, kind="ExternalInput")
    wq = nc.dram_tensor("wq", [128, 16 * 512], BF16, kind="ExternalInput")
    bq = nc.dram_tensor("bq", [128, 4], F32, kind="ExternalInput")
    wk = nc.dram_tensor("wk", [128, 2048], BF16, kind="ExternalInput")
    bk = nc.dram_tensor("bk", [128, 1], F32, kind="ExternalInput")
    wv = nc.dram_tensor("wv", [128, 2048], BF16, kind="ExternalInput")
    bv = nc.dram_tensor("bv", [128, 1], F32, kind="ExternalInput")
    wo = nc.dram_tensor("wo", [128, 16 * 512], BF16, kind="ExternalInput")
    bo = nc.dram_tensor("bo", [1, 512], BF16, kind="ExternalInput")
    band = nc.dram_tensor("band", [128, nband * 512], BF16, kind="ExternalInput")
    ones = nc.dram_tensor("ones", [128, 128], BF16, kind="ExternalInput")
    ident = nc.dram_tensor("ident", [128, 128], BF16, kind="ExternalInput")
    out_ext = nc.dram_tensor("out", [L, 512], F32, kind="ExternalOutput")

    ag_warm_in = nc.dram_tensor("ag_warm_in", [256, 512], BF16)
    ag_warm_out = nc.dram_tensor("ag_warm_out", [1024, 512], BF16)
    ag_in = [[nc.dram_tensor(f"ag_in{j}_{g}", [256, 512], BF16)
              for g in range(2)] for j in range(NJ)]
    ag_out = [[nc.dram_tensor(f"ag_out{j}_{g}", [1024, 512], BF16)
               for g in range(2)] for j in range(NJ)]
    groups = [[0, 1, 2, 3], [4, 5, 6, 7]]

    with tile.TileContext(nc) as tc:
        with tc.tile_pool(name="persist", bufs=1) as persist:
            ones_sb = persist.tile([128, 128], BF16, tag="ones")
            nc.sync.dma_start(ones_sb[:], ones[:])
            # weights needed first: wk/wv lead the scalar queue
            wk_sb = persist.tile([128, 2048], BF16, tag="wk")
            nc.scalar.dma_start(wk_sb[:], wk[:])
            wv_sb = persist.tile([128, 2048], BF16, tag="wv")
            nc.scalar.dma_start(wv_sb[:], wv[:])
            # warm up the collective path with the REAL transfer shape so the
            # first in-loop AllGather doesn't pay the plan-setup cost
            nc.gpsimd.dma_start(ag_warm_in[0:128, 0:128], ones[:])
            nc.gpsimd.dma_start(ag_warm_in[128:256, 0:128], ones[:])
            nc.gpsimd.collective_compute(
                "AllGather", mybir.AluOpType.bypass,
                replica_groups=groups,
                ins=[ag_warm_in[:]], outs=[ag_warm_out[:]])

            ident_sb = persist.tile([128, 128], BF16, tag="ident")
            nc.scalar.dma_start(ident_sb[:], ident[:])
            bq_sb = persist.tile([128, 4], F32, tag="bq")
            nc.scalar.dma_start(bq_sb[:], bq[:])
            bk_sb = persist.tile([128, 1], F32, tag="bk")
            nc.scalar.dma_start(bk_sb[:], bk[:])
            bv_sb = persist.tile([128, 1], F32, tag="bv")
            nc.scalar.dma_start(bv_sb[:], bv[:])
            bo_sb = persist.tile([1, 512], BF16, tag="bo")
            nc.scalar.dma_start(bo_sb[:], bo[:])
            wq_sb = persist.tile([128, 16 * 512], BF16, tag="wq")
            nc.scalar.dma_start(wq_sb[:], wq[:])
            wo_sb = persist.tile([128, 16 * 512], BF16, tag="wo")
            band_sb = persist.tile([128, nband * 512], BF16, tag="band")

            kT_sb = persist.tile([128, L], BF16, tag="kT")
            v_sb = [persist.tile([128, 130], BF16, tag=f"v{t}", name=f"v{t}")
                    for t in range(NI)]
            for t in range(NI):
                nc.vector.memset(v_sb[t][:, 64:65], 1.0)
                nc.vector.memset(v_sb[t][:, 129:130], 1.0)
            QT_sb = [persist.tile([128, L], BF16, tag=f"qt{m}", name=f"qt{m}")
                     for m in range(NPAIR)]
            # gathered attnT blocks, filled per (j, g2) as AllGathers complete
            ag_sb = [persist.tile([128, L], BF16, tag=f"ag{kk}", name=f"ag{kk}")
                     for kk in range(16)]

            # ---- KV projection: kT and vT streamed over 16 row chunks
            with tc.tile_pool(name="kv_era", bufs=1) as kv_era:
                vT_sb = kv_era.tile([128, L], BF16, tag="vT")
                with (
                    tc.tile_pool(name="kv_stream", bufs=4) as kv_stream,
                    tc.tile_pool(name="kv_psum", bufs=1, space="PSUM") as kv_psum,
                ):
                    psk = kv_psum.tile([128, L], F32, tag="psk")
                    psv = kv_psum.tile([128, L], F32, tag="psv")
                    for k in range(NCHUNK):
                        ch = kv_stream.tile([128, L], BF16, tag="kvch")
                        eng = nc.sync if k % 2 == 0 else nc.scalar
                        # two half-row DMAs so matmuls start on the first
                        # half while the second is still in flight
                        eng.dma_start(ch[:, 0:1024],
                                      kvT[128 * k:128 * (k + 1), 0:1024])
                        eng.dma_start(ch[:, 1024:2048],
                                      kvT[128 * k:128 * (k + 1), 1024:2048])
                        for jj in range(4):
                            nc.tensor.matmul(
                                psk[:, 512 * jj:512 * (jj + 1)],
                                wk_sb[:, 128 * k:128 * (k + 1)],
                                ch[:, 512 * jj:512 * (jj + 1)],
                                start=(k == 0), stop=(k == NCHUNK - 1),
                                skip_group_check=True)
                        for jj in range(4):
                            nc.tensor.matmul(
                                psv[:, 512 * jj:512 * (jj + 1)],
                                wv_sb[:, 128 * k:128 * (k + 1)],
                                ch[:, 512 * jj:512 * (jj + 1)],
                                start=(k == 0), stop=(k == NCHUNK - 1),
                                skip_group_check=True)
                    for jj in range(4):
                        nc.scalar.activation(
                            kT_sb[:, 512 * jj:512 * (jj + 1)],
                            psk[:, 512 * jj:512 * (jj + 1)],
                            AF.Identity, bias=bk_sb[:])
                        nc.scalar.activation(
                            vT_sb[:, 512 * jj:512 * (jj + 1)],
                            psv[:, 512 * jj:512 * (jj + 1)],
                            AF.Identity, bias=bv_sb[:])

                # transpose vT -> v tiles [pos, dim] (TensorE, exact)
                with tc.tile_pool(name="tr_psum", bufs=2, space="PSUM") as trp:
                    for t in range(NI):
                        tp = trp.tile([128, 128], BF16, tag="tp")
                        nc.tensor.transpose(
                            tp[:], vT_sb[:, 128 * t:128 * (t + 1)], ident_sb[:])
                        nc.vector.tensor_copy(v_sb[t][:, 0:64], tp[:, 0:64])
                        nc.vector.tensor_copy(v_sb[t][:, 65:129], tp[:, 64:128])

            # ---- Q projection (two 1024-col halves, 8 PSUM banks each)
            with (
                tc.tile_pool(name="q_stream", bufs=4) as q_stream,
                tc.tile_pool(name="q_psum", bufs=1, space="PSUM") as q_psum,
            ):
                for jp in range(2):
                    psq = [q_psum.tile([128, 1024], F32, tag=f"psq{m}",
                                       name=f"psq{m}") for m in range(4)]
                    for k in range(NCHUNK):
                        ch = q_stream.tile([128, 1024], BF16, tag="qch")
                        eng = nc.sync if k % 2 == 0 else nc.scalar
                        eng.dma_start(
                            ch[:], qT[128 * k:128 * (k + 1),
                                      1024 * jp:1024 * (jp + 1)])
                        for m in range(4):
                            for jj in range(2):
                                nc.tensor.matmul(
                                    psq[m][:, 512 * jj:512 * (jj + 1)],
                                    wq_sb[:, 512 * k + 128 * m:
                                          512 * k + 128 * (m + 1)],
                                    ch[:, 512 * jj:512 * (jj + 1)],
                                    start=(k == 0), stop=(k == NCHUNK - 1),
                                    skip_group_check=True)
                    for m in range(4):
                        # split the final writeback between ACT and DVE so
                        # the PSUM pool hands off to the attention pools
                        # ~2x sooner after the last matmul
                        if jp == 1 and m >= 2:
                            nc.vector.tensor_scalar_add(
                                QT_sb[m][:, 1024 * jp:1024 * (jp + 1)],
                                psq[m][:], bq_sb[:, m:m + 1])
                        else:
                            for jj in range(2):
                                j = 2 * jp + jj
                                nc.scalar.activation(
                                    QT_sb[m][:, 512 * j:512 * (j + 1)],
                                    psq[m][:, 512 * jj:512 * (jj + 1)],
                                    AF.Identity, bias=bq_sb[:, m:m + 1])



            # weights for the attention era: band first (needed at j0),
            # wo later (needed from the first oproj unit)
            nc.scalar.dma_start(band_sb[:], band[:])
            nc.scalar.dma_start(wo_sb[:], wo[:])

            # ---- Attention (j-outer, pairs in order [2,3,0,1]) with the
            # ---- output projection interleaved as half-accumulations
            with (
                tc.tile_pool(name="pt_pool", bufs=3) as pt_pool,
                tc.tile_pool(name="at_pool", bufs=3) as at_pool,
                tc.tile_pool(name="rc_pool", bufs=2) as rc_pool,
                tc.tile_pool(name="o_out", bufs=2) as o_out,
                tc.tile_pool(name="qk_psum", bufs=2, space="PSUM") as qk_psum,
                tc.tile_pool(name="pv_psum", bufs=1, space="PSUM") as pv_psum,
                tc.tile_pool(name="o_psum", bufs=2, space="PSUM") as o_psum,
            ):
                pending_scatters = []

                def _emit_scatter(j_, g2_):
                    for rp in range(TPR):
                        for q01 in range(2):
                            kk = 8 * g2_ + 2 * rp + q01
                            nc.sync.dma_start(
                                ag_sb[kk][:, 512 * j_:512 * (j_ + 1)],
                                ag_out[j_][g2_][
                                    256 * rp + 128 * q01:
                                    256 * rp + 128 * (q01 + 1), :])

                def flush_scatters(now_blk):
                    # emit scatter DMAs only once the AllGather has had ~2
                    # pair-blocks to complete, so they never head-of-line
                    # block the sync queue for the ag_in writes behind them
                    while pending_scatters and (
                            now_blk is None
                            or pending_scatters[0][2] + 2 <= now_blk):
                        j_, g2_, _ = pending_scatters.pop(0)
                        _emit_scatter(j_, g2_)

                def ensure_scatter(j_, g2_):
                    for e in list(pending_scatters):
                        if e[0] == j_ and e[1] == g2_:
                            pending_scatters.remove(e)
                            _emit_scatter(j_, g2_)

                def attn_block(j, pr):
                    lv = live[j]
                    pva = pv_psum.tile([65, 512], F32, tag="pva")
                    pvb = pv_psum.tile([65, 512], F32, tag="pvb")
                    for n, i in enumerate(lv):
                        ps = qk_psum.tile([128, 1024], F32, tag="qk")
                        nc.tensor.matmul(
                            ps[:, 0:512],
                            kT_sb[0:64, 128 * i:128 * (i + 1)],
                            QT_sb[pr][0:64, 512 * j:512 * (j + 1)],
                            start=True, stop=True, skip_group_check=True)
                        nc.tensor.matmul(
                            ps[:, 512:1024],
                            kT_sb[64:128, 128 * i:128 * (i + 1)],
                            QT_sb[pr][64:128, 512 * j:512 * (j + 1)],
                            start=True, stop=True, skip_group_check=True)
                        pt = pt_pool.tile([128, 1024], BF16, tag="pt")
                        nc.scalar.activation(pt[:], ps[:], AF.Exp)
                        if (j, i) in band_idx:
                            nb = band_idx[(j, i)]
                            bt = band_sb[:, 512 * nb:512 * (nb + 1)]
                            nc.vector.tensor_mul(pt[:, 0:512], pt[:, 0:512], bt)
                            nc.vector.tensor_mul(pt[:, 512:1024],
                                                 pt[:, 512:1024], bt)
                        nc.tensor.matmul(
                            pva[:], v_sb[i][:, 0:65], pt[:, 0:512],
                            start=(n == 0), stop=(n == len(lv) - 1),
                            skip_group_check=True)
                        nc.tensor.matmul(
                            pvb[:], v_sb[i][:, 65:130], pt[:, 512:1024],
                            start=(n == 0), stop=(n == len(lv) - 1),
                            skip_group_check=True)
                    # copy PV results out of PSUM promptly (DVE) so the pv
                    # banks free up for the next pair regardless of gpsimd
                    au = at_pool.tile([65, 1024], BF16, tag="au")
                    nc.vector.tensor_copy(au[:, 0:512], pva[:])
                    nc.vector.tensor_copy(au[:, 512:1024], pvb[:])
                    # normalization: 1/denominator, broadcast, multiply.
                    # Keep everything bf16 so the DVE muls hit the 2x mode.
                    rsa = rc_pool.tile([1, 512], F32, tag="rsa")
                    rsb = rc_pool.tile([1, 512], F32, tag="rsb")
                    nc.vector.tensor_copy(rsa[:], au[64:65, 0:512])
                    nc.vector.tensor_copy(rsb[:], au[64:65, 512:1024])
                    rra = rc_pool.tile([1, 512], F32, tag="rra")
                    rrb = rc_pool.tile([1, 512], F32, tag="rrb")
                    nc.vector.reciprocal_approx_fast(out=rra[:], in_=rsa[:])
                    nc.vector.reciprocal_approx_fast(out=rrb[:], in_=rsb[:])
                    rh = rc_pool.tile([1, 1024], BF16, tag="rh")
                    nc.vector.tensor_copy(rh[:, 0:512], rra[:])
                    nc.vector.tensor_copy(rh[:, 512:1024], rrb[:])
                    rb = rc_pool.tile([64, 1024], BF16, tag="rb")
                    nc.gpsimd.partition_broadcast(rb[:, 0:512], rh[:, 0:512])
                    nc.gpsimd.partition_broadcast(rb[:, 512:1024],
                                                  rh[:, 512:1024])
                    ata = at_pool.tile([64, 1024], BF16, tag="ata")
                    nc.vector.tensor_mul(ata[:, 0:512], au[0:64, 0:512],
                                         rb[:, 0:512])
                    nc.vector.tensor_mul(ata[:, 512:1024],
                                         au[0:64, 512:1024], rb[:, 512:1024])
                    g2, p01 = pr // 2, pr % 2
                    nc.sync.dma_start(
                        ag_in[j][g2][128 * p01:128 * p01 + 64, :],
                        ata[:, 0:512])
                    nc.sync.dma_start(
                        ag_in[j][g2][128 * p01 + 64:128 * (p01 + 1), :],
                        ata[:, 512:1024])
                    if p01 == 1:
                        nc.gpsimd.collective_compute(
                            "AllGather", mybir.AluOpType.bypass,
                            replica_groups=groups,
                            ins=[ag_in[j][g2][:]], outs=[ag_out[j][g2][:]])
                        pending_scatters.append((j, g2, cur_blk[0]))

                pso_map = {}

                def oproj_half(t, half):
                    # half 0: bias + gathered g2=1 blocks (kk 8..15);
                    # half 1: g2=0 blocks (kk 0..7), then writeback.
                    ensure_scatter(t // 4, 1 if half == 0 else 0)
                    if half == 0:
                        pso = o_psum.tile([128, 512], F32, tag="pso")
                        pso_map[t] = pso
                        nc.tensor.matmul(pso[:], ones_sb[0:1, :], bo_sb[:],
                                         start=True, stop=False,
                                         skip_group_check=True)
                        for kk in range(8, 16):
                            nc.tensor.matmul(
                                pso[:], ag_sb[kk][:, 128 * t:128 * (t + 1)],
                                wo_sb[:, 512 * kk:512 * (kk + 1)],
                                start=False, stop=False, skip_group_check=True)
                    else:
                        pso = pso_map.pop(t)
                        for kk in range(8):
                            nc.tensor.matmul(
                                pso[:], ag_sb[kk][:, 128 * t:128 * (t + 1)],
                                wo_sb[:, 512 * kk:512 * (kk + 1)],
                                start=False, stop=(kk == 7),
                                skip_group_check=True)
                        osb = o_out.tile([128, 512], F32, tag="osb")
                        nc.vector.tensor_copy(osb[:], pso[:])
                        nc.sync.dma_start(out_ext[128 * t:128 * (t + 1), :],
                                          osb[:])

                PR_ORDER = [2, 3, 0, 1]
                oq = []   # entries: [t, next_half, ready0, ready1]
                cur_blk = [0]

                def pop_ready(blk_, budget):
                    # drain up to `budget` half-units.  Entries may be taken
                    # out of FIFO order (a ready h0 can jump a cc-gated h1),
                    # but at most 2 tiles may hold an o_psum slot (bufs=2).
                    while budget > 0:
                        n_open = sum(1 for e in oq if e[1] == 1)
                        done = False
                        for e in oq:
                            rdy = e[2] if e[1] == 0 else e[3]
                            if blk_ is not None and (rdy is None or rdy > blk_):
                                continue
                            if e[1] == 0 and n_open >= 2:
                                continue
                            oproj_half(e[0], e[1])
                            if e[1] == 0:
                                e[1] = 1
                            else:
                                oq.remove(e)
                            budget -= 1
                            done = True
                            break
                        if not done:
                            break
                for j in range(NJ):
                    for pr in PR_ORDER:
                        flush_scatters(cur_blk[0])
                        attn_block(j, pr)
                        cur_blk[0] += 1
                        blk = cur_blk[0]
                        if pr == 3:   # g2=1 AllGather just issued
                            slack = 2 if j == NJ - 1 else 4
                            for t in range(4 * j, 4 * j + 4):
                                oq.append([t, 0, blk + slack, None])
                        if pr == 1:   # g2=0 AllGather issued (chunk done)
                            for e in oq:
                                if 4 * j <= e[0] < 4 * j + 4:
                                    e[3] = blk + 4
                        pop_ready(blk, 2)
                flush_scatters(None)
                pop_ready(None, len(oq) * 2 + 4)

    nc.compile()
    _graph_cache[key] = nc
    return nc


def _pack16(a):
    """[2048, X] -> [128, 16*X] with row-chunk k at cols [X*k, X*(k+1))."""
    x = a.shape[1]
    return np.ascontiguousarray(
        a.reshape(16, 128, x).transpose(1, 0, 2).reshape(128, 16 * x))


def kernel(query, kv, Wq, bq, Wkv, bkv, Wo, bo, attn_mask, key_padding_mask):
    global last_results
    query = np.asarray(query, np.float32)
    kv = np.asarray(kv, np.float32)
    Wq = np.asarray(Wq, np.float32)
    bq = np.asarray(bq, np.float32)
    Wkv = np.asarray(Wkv, np.float32)
    bkv = np.asarray(bkv, np.float32)
    Wo = np.asarray(Wo, np.float32)
    bo = np.asarray(bo, np.float32)
    attn_mask = np.asarray(attn_mask, np.float32)
    kpm = np.asarray(key_padding_mask)

    eff = [attn_mask + np.where(kpm[b], np.float32(-1e9), np.float32(0.0))[None, :]
           for b in range(B)]
    live, band_list = _classify_blocks(eff)
    live_key = tuple((j, tuple(lv)) for j, lv in sorted(live.items()))
    band_key = tuple(band_list)

    nc = _build_graph(live_key, band_key)

    # Host-side shard prep (bf16 for all TensorEngine operands)
    qTh = [np.ascontiguousarray(query[b].T.astype(NPBF16)) for b in range(B)]
    kvTh = [np.ascontiguousarray(kv[b].T.astype(NPBF16)) for b in range(B)]
    bandh = []
    with np.errstate(over="ignore", under="ignore"):
        for b in range(B):
            if band_list:
                bt = np.stack(
                    [np.exp(eff[b][j * LQC:(j + 1) * LQC,
                                   i * KT:(i + 1) * KT].T)
                     for (j, i) in band_list]).astype(NPBF16)
                bandh.append(np.ascontiguousarray(
                    bt.transpose(1, 0, 2).reshape(KT, len(band_list) * LQC)))
            else:
                bandh.append(np.zeros((KT, 512), NPBF16))
    ones_h = np.ones((128, 128), NPBF16)
    ident_h = np.eye(128, dtype=NPBF16)

    Wq_h = Wq.reshape(HID, NH, D)
    bq_h = bq.reshape(NH, D)
    Wo_h = Wo.reshape(NH, D, HID)

    in_maps = []
    for c in range(N_CORES):
        b, r = c // TPR, c % TPR
        heads_q = [8 * r + pr + 4 * e for pr in range(NPAIR) for e in range(2)]
        perm_glob = [8 * rp + (2 * g2 + p01) + 4 * e
                     for g2 in range(2) for rp in range(TPR)
                     for p01 in range(2) for e in range(2)]
        wq_c = _pack16(
            (Wq_h[:, heads_q, :].reshape(HID, 512) * SCALE).astype(NPBF16))
        bq_c = np.ascontiguousarray(
            (bq_h[heads_q].reshape(512) * SCALE).reshape(4, 128).T)
        wk_c = _pack16(Wkv[:, 128 * r:128 * (r + 1)].astype(NPBF16))
        bk_c = np.ascontiguousarray(bkv[128 * r:128 * (r + 1)])[:, None]
        wv_c = _pack16(Wkv[:, 512 + 128 * r:512 + 128 * (r + 1)].astype(NPBF16))
        bv_c = np.ascontiguousarray(
            bkv[512 + 128 * r:512 + 128 * (r + 1)])[:, None]
        wo_c = _pack16(
            Wo_h[perm_glob].reshape(HID, HID)[:, 512 * r:512 * (r + 1)]
            .astype(NPBF16))
        bo_c = np.ascontiguousarray(
            bo[512 * r:512 * (r + 1)].astype(NPBF16))[None, :]
        in_maps.append({
            "qT": qTh[b], "kvT": kvTh[b],
            "wq": wq_c, "bq": bq_c,
            "wk": wk_c, "bk": bk_c,
            "wv": wv_c, "bv": bv_c,
            "wo": wo_c, "bo": bo_c,
            "band": bandh[b], "ones": ones_h, "ident": ident_h,
        })

    last_results = run_bass_kernel_spmd(nc, in_maps, core_ids=list(range(N_CORES)))

    out = np.empty((B, L, HID), np.float32)
    for c in range(N_CORES):
        b, r = c // TPR, c % TPR
        out[b, :, 512 * r:512 * (r + 1)] = last_results.results[c]["out"]
    return out


# revision 42
# speedup vs baseline: 1.1349x; 1.0250x over previous
"""GQA attention (B=2, L=2048, HID=2048, 32 Q heads / 8 KV heads) on 8 TRN2 cores.

Sharding: data-parallel on batch (2) x tensor-parallel on heads (4).
Core c: batch b = c//4, TP rank r = c%4 owns q heads {8r..8r+7} (whole GQA
groups: kv heads 2r, 2r+1). Compute in bf16 on the TensorEngine (fp32 PSUM
accumulation), fp32 softmax statistics. Per-core pipeline:
  1. kT = Wk_c.T @ kv_b.T and vT = Wv_c.T @ kv_b.T streamed over 16 row
     chunks of kvT; vT transposed back to [pos, dim] tiles via TensorE.
  2. QT = (Wq_c * scale).T @ query_b.T -> [512, L] bf16 (pair-major rows).
  3. per head-pair (g0-head, g1-head) and q-chunk j: scores^T = kT.T @ QT
     (two row-packed K=64 matmuls), exp on ACT, multiplicative mask
     (host-exp'd band tiles) on DVE, PV^T with ones-column giving the
     softmax denominator in PSUM row 64. PV results are copied out of PSUM
     immediately on DVE (releasing banks); normalization (reciprocal +
     gpsimd partition broadcast + mul) happens off the critical path.
  4. AllGather attnT (bf16) per (q-chunk, pair-group) over the 4-rank TP
     group.  Collectives are the ONLY thing on the gpsimd queue besides
     broadcasts, so their serialization does not stall the attention pipe.
  5. out_c[:, 512r:+512] = attnT_full.T @ Wo_perm_c + bo_c, split into two
     8-matmul half-accumulations per 128-row tile, interleaved into the
     attention instruction stream with enough slack to cover AG latency.
Host assembles [2, 2048, 2048] from per-core [2048, 512] f32 slabs.

Mask handling is input-driven: the effective additive mask (attn_mask +
key-padding) is classified on host per (q-chunk, k-tile) block as
all-masked (skip), all-zero (no op), or band (exp(mask) shipped and
multiplied into exp(scores)).
"""

import numpy as np
import ml_dtypes
import concourse.bass as bass
import concourse.mybir as mybir
import concourse.tile as tile
from concourse import bacc
from concourse.bass_utils import run_bass_kernel_spmd

F32 = mybir.dt.float32
BF16 = mybir.dt.bfloat16
AF = mybir.ActivationFunctionType
NPBF16 = ml_dtypes.bfloat16

B, L, HID = 2, 2048, 2048
NH, D, NKV = 32, 64, 8
SCALE = 0.125
N_CORES = 8
TPR = 4          # TP ranks per batch group
NPAIR = 4        # head pairs per core (g0-head, g1-head)
LQC = 512        # Lq chunk for attention (PSUM-bank sized)
NJ = L // LQC    # 4
KT = 128         # k-position tile
NI = L // KT     # 16
NEG_THRESH = -1.0e8
NCHUNK = HID // 128  # 16

_graph_cache = {}
last_results = None  # BassKernelResults of the most recent run (for test harness)


def _classify_blocks(eff_masks):
    """eff_masks: list of B arrays [L, L] (q, k). Returns (live, band_list)
    where live[j] is the ascending list of k-tiles to compute for q-chunk j and
    band_list orders the (j, i) blocks that need explicit mask values."""
    live = {}
    band_list = []
    for j in range(NJ):
        lv = []
        for i in range(NI):
            subs = [m[j * LQC:(j + 1) * LQC, i * KT:(i + 1) * KT] for m in eff_masks]
            if all((s <= NEG_THRESH).all() for s in subs):
                continue  # fully masked in every batch: contributes exactly 0
            lv.append(i)
            if not all((s == 0.0).all() for s in subs):
                band_list.append((j, i))
        live[j] = lv
    return live, band_list


def _build_graph(live_key, band_key):
    key = (live_key, band_key)
    if key in _graph_cache:
        return _graph_cache[key]

    live = {j: list(lv) for j, lv in live_key}
    band_list = list(band_key)
    band_idx = {ji: n for n, ji in enumerate(band_list)}
    nband = max(1, len(band_list))

    nc = bacc.Bacc("TRN2", target_bir_lowering=False, debug=False,
                   num_devices=N_CORES)

    qT = nc.dram_tensor("qT", [HID, L], BF16, kind="ExternalInput")
    kvT = nc.dram_tensor("kvT", [HID, L], BF16, kind="ExternalInput")
    wq = nc.dram_tensor("wq", [128, 16 * 512], BF16, kind="ExternalInput")
    bq = nc.dram_tensor("bq", [128, 4], F32, kind="ExternalInput")
    wk = nc.dram_tensor("wk", [128, 2048], BF16, kind="ExternalInput")
    bk = nc.dram_tensor("bk", [128, 1], F32, kind="ExternalInput")
    wv = nc.dram_tensor("wv", [128, 2048], BF16, kind="ExternalInput")
    bv = nc.dram_tensor("bv", [128, 1], F32, kind="ExternalInput")
    wo = nc.dram_tensor("wo", [128, 16 * 512], BF16, kind="ExternalInput")
    bo = nc.dram_tensor("bo", [1, 512], BF16, kind="ExternalInput")
    band = nc.dram_tensor("band", [128, nband * 512], BF16, kind="ExternalInput")
    ones = nc.dram_tensor("ones", [128, 128], BF16, kind="ExternalInput")
    ident = nc.dram_tensor("ident", [128, 128], BF16, kind="ExternalInput")
    out_ext = nc.dram_tensor("out", [L, 512], F32, kind="ExternalOutput")

    ag_warm_in = nc.dram_tensor("ag_warm_in", [256, 512], BF16)
    ag_warm_out = nc.dram_tensor("ag_warm_out", [1024, 512], BF16)
    ag_in = [[nc.dram_tensor(f"ag_in{j}_{g}", [256, 512], BF16)
              for g in range(2)] for j in range(NJ)]
    ag_out = [[nc.dram_tensor(f"ag_out{j}_{g}", [1024, 512], BF16)
               for g in range(2)] for j in range(NJ)]
    groups = [[0, 1, 2, 3], [4, 5, 6, 7]]

    with tile.TileContext(nc) as tc:
        with tc.tile_pool(name="persist", bufs=1) as persist:
            ones_sb = persist.tile([128, 128], BF16, tag="ones")
            nc.sync.dma_start(ones_sb[:], ones[:])
            # weights needed first: wk/wv lead the scalar queue
            wk_sb = persist.tile([128, 2048], BF16, tag="wk")
            nc.scalar.dma_start(wk_sb[:], wk[:])
            wv_sb = persist.tile([128, 2048], BF16, tag="wv")
            nc.scalar.dma_start(wv_sb[:], wv[:])
            # warm up the collective path with the REAL transfer shape so the
            # first in-loop AllGather doesn't pay the plan-setup cost
            nc.gpsimd.dma_start(ag_warm_in[0:128, 0:128], ones[:])
            nc.gpsimd.dma_start(ag_warm_in[128:256, 0:128], ones[:])
            nc.gpsimd.collective_compute(
                "AllGather", mybir.AluOpType.bypass,
                replica_groups=groups,
                ins=[ag_warm_in[:]], outs=[ag_warm_out[:]])

            ident_sb = persist.tile([128, 128], BF16, tag="ident")
            nc.scalar.dma_start(ident_sb[:], ident[:])
            bq_sb = persist.tile([128, 4], F32, tag="bq")
            nc.scalar.dma_start(bq_sb[:], bq[:])
            bk_sb = persist.tile([128, 1], F32, tag="bk")
            nc.scalar.dma_start(bk_sb[:], bk[:])
            bv_sb = persist.tile([128, 1], F32, tag="bv")
            nc.scalar.dma_start(bv_sb[:], bv[:])
            bo_sb = persist.tile([1, 512], BF16, tag="bo")
            nc.scalar.dma_start(bo_sb[:], bo[:])
            wq_sb = persist.tile([128, 16 * 512], BF16, tag="wq")
            nc.scalar.dma_start(wq_sb[:], wq[:])
            wo_sb = persist.tile([128, 16 * 512], BF16, tag="wo")
            band_sb = persist.tile([128, nband * 512], BF16, tag="band")

            kT_sb = persist.tile([128, L], BF16, tag="kT")
            v_sb = [persist.tile([128, 130], BF16, tag=f"v{t}", name=f"v{t}")
                    for t in range(NI)]
            for t in range(NI):
                nc.vector.memset(v_sb[t][:, 64:65], 1.0)
                nc.vector.memset(v_sb[t][:, 129:130], 1.0)
            QT_sb = [persist.tile([128, L], BF16, tag=f"qt{m}", name=f"qt{m}")
                     for m in range(NPAIR)]
            # gathered attnT blocks, filled per (j, g2) as AllGathers complete
            ag_sb = [persist.tile([128, L], BF16, tag=f"ag{kk}", name=f"ag{kk}")
                     for kk in range(16)]

            # ---- KV projection: kT and vT streamed over 16 row chunks
            with tc.tile_pool(name="kv_era", bufs=1) as kv_era:
                vT_sb = kv_era.tile([128, L], BF16, tag="vT")
                with (
                    tc.tile_pool(name="kv_stream", bufs=4) as kv_stream,
                    tc.tile_pool(name="kv_psum", bufs=1, space="PSUM") as kv_psum,
                ):
                    psk = kv_psum.tile([128, L], F32, tag="psk")
                    psv = kv_psum.tile([128, L], F32, tag="psv")
                    for k in range(NCHUNK):
                        ch = kv_stream.tile([128, L], BF16, tag="kvch")
                        # the whole kvT stream rides the sync ring so no
                        # chunk ever queues behind the weight loads on the
                        # scalar ring; two half-row DMAs so matmuls start
                        # on the first half while the second is in flight
                        nc.sync.dma_start(ch[:, 0:1024],
                                          kvT[128 * k:128 * (k + 1), 0:1024])
                        nc.sync.dma_start(ch[:, 1024:2048],
                                          kvT[128 * k:128 * (k + 1),
                                              1024:2048])
                        for jj in range(4):
                            nc.tensor.matmul(
                                psk[:, 512 * jj:512 * (jj + 1)],
                                wk_sb[:, 128 * k:128 * (k + 1)],
                                ch[:, 512 * jj:512 * (jj + 1)],
                                start=(k == 0), stop=(k == NCHUNK - 1),
                                skip_group_check=True)
                        for jj in range(4):
                            nc.tensor.matmul(
                                psv[:, 512 * jj:512 * (jj + 1)],
                                wv_sb[:, 128 * k:128 * (k + 1)],
                                ch[:, 512 * jj:512 * (jj + 1)],
                                start=(k == 0), stop=(k == NCHUNK - 1),
                                skip_group_check=True)
                    for jj in range(4):
                        nc.scalar.activation(
                            kT_sb[:, 512 * jj:512 * (jj + 1)],
                            psk[:, 512 * jj:512 * (jj + 1)],
                            AF.Identity, bias=bk_sb[:])
                        nc.scalar.activation(
                            vT_sb[:, 512 * jj:512 * (jj + 1)],
                            psv[:, 512 * jj:512 * (jj + 1)],
                            AF.Identity, bias=bv_sb[:])

                # transpose vT -> v tiles [pos, dim] (TensorE, exact)
                with tc.tile_pool(name="tr_psum", bufs=2, space="PSUM") as trp:
                    for t in range(NI):
                        tp = trp.tile([128, 128], BF16, tag="tp")
                        nc.tensor.transpose(
                            tp[:], vT_sb[:, 128 * t:128 * (t + 1)], ident_sb[:])
                        nc.vector.tensor_copy(v_sb[t][:, 0:64], tp[:, 0:64])
                        nc.vector.tensor_copy(v_sb[t][:, 65:129], tp[:, 64:128])

            # ---- Q projection (two 1024-col halves, 8 PSUM banks each)
            with (
                tc.tile_pool(name="q_stream", bufs=4) as q_stream,
                tc.tile_pool(name="q_psum", bufs=1, space="PSUM") as q_psum,
            ):
                for jp in range(2):
                    psq = [q_psum.tile([128, 1024], F32, tag=f"psq{m}",
                                       name=f"psq{m}") for m in range(4)]
                    for k in range(NCHUNK):
                        ch = q_stream.tile([128, 1024], BF16, tag="qch")
                        eng = nc.sync if k % 2 == 0 else nc.scalar
                        eng.dma_start(
                            ch[:], qT[128 * k:128 * (k + 1),
                                      1024 * jp:1024 * (jp + 1)])
                        for m in range(4):
                            for jj in range(2):
                                nc.tensor.matmul(
                                    psq[m][:, 512 * jj:512 * (jj + 1)],
                                    wq_sb[:, 512 * k + 128 * m:
                                          512 * k + 128 * (m + 1)],
                                    ch[:, 512 * jj:512 * (jj + 1)],
                                    start=(k == 0), stop=(k == NCHUNK - 1),
                                    skip_group_check=True)
                    for m in range(4):
                        # split the final writeback between ACT and DVE so
                        # the PSUM pool hands off to the attention pools
                        # ~2x sooner after the last matmul
                        if jp == 1 and m >= 2:
                            nc.vector.tensor_scalar_add(
                                QT_sb[m][:, 1024 * jp:1024 * (jp + 1)],
                                psq[m][:], bq_sb[:, m:m + 1])
                        else:
                            for jj in range(2):
                                j = 2 * jp + jj
                                nc.scalar.activation(
                                    QT_sb[m][:, 512 * j:512 * (j + 1)],
                                    psq[m][:, 512 * jj:512 * (jj + 1)],
                                    AF.Identity, bias=bq_sb[:, m:m + 1])



            # weights for the attention era: band first (needed at j0),
            # wo later (needed from the first oproj unit)
            nc.scalar.dma_start(band_sb[:], band[:])
            nc.scalar.dma_start(wo_sb[:], wo[:])

            # ---- Attention (j-outer, pairs in order [2,3,0,1]) with the
            # ---- output projection interleaved as half-accumulations
            with (
                tc.tile_pool(name="pt_pool", bufs=4) as pt_pool,
                tc.tile_pool(name="at_pool", bufs=3) as at_pool,
                tc.tile_pool(name="rc_pool", bufs=2) as rc_pool,
                tc.tile_pool(name="o_out", bufs=2) as o_out,
                tc.tile_pool(name="qk_psum", bufs=2, space="PSUM") as qk_psum,
                tc.tile_pool(name="pv_psum", bufs=1, space="PSUM") as pv_psum,
                tc.tile_pool(name="o_psum", bufs=2, space="PSUM") as o_psum,
            ):
                pending_scatters = []

                def _emit_scatter(j_, g2_):
                    for rp in range(TPR):
                        for q01 in range(2):
                            kk = 8 * g2_ + 2 * rp + q01
                            nc.sync.dma_start(
                                ag_sb[kk][:, 512 * j_:512 * (j_ + 1)],
                                ag_out[j_][g2_][
                                    256 * rp + 128 * q01:
                                    256 * rp + 128 * (q01 + 1), :])

                def flush_scatters(now_blk):
                    # emit scatter DMAs only once the AllGather has had ~2
                    # pair-blocks to complete, so they never head-of-line
                    # block the sync queue for the ag_in writes behind them
                    while pending_scatters and (
                            now_blk is None
                            or pending_scatters[0][2] + 2 <= now_blk):
                        j_, g2_, _ = pending_scatters.pop(0)
                        _emit_scatter(j_, g2_)

                def ensure_scatter(j_, g2_):
                    for e in list(pending_scatters):
                        if e[0] == j_ and e[1] == g2_:
                            pending_scatters.remove(e)
                            _emit_scatter(j_, g2_)

                def attn_block(j, pr):
                    lv = live[j]
                    pva = pv_psum.tile([65, 512], F32, tag="pva")
                    pvb = pv_psum.tile([65, 512], F32, tag="pvb")
                    for n, i in enumerate(lv):
                        ps = qk_psum.tile([128, 1024], F32, tag="qk")
                        nc.tensor.matmul(
                            ps[:, 0:512],
                            kT_sb[0:64, 128 * i:128 * (i + 1)],
                            QT_sb[pr][0:64, 512 * j:512 * (j + 1)],
                            start=True, stop=True, skip_group_check=True)
                        nc.tensor.matmul(
                            ps[:, 512:1024],
                            kT_sb[64:128, 128 * i:128 * (i + 1)],
                            QT_sb[pr][64:128, 512 * j:512 * (j + 1)],
                            start=True, stop=True, skip_group_check=True)
                        pt = pt_pool.tile([128, 1024], BF16, tag="pt")
                        nc.scalar.activation(pt[:], ps[:], AF.Exp)
                        if (j, i) in band_idx:
                            nb = band_idx[(j, i)]
                            bt = band_sb[:, 512 * nb:512 * (nb + 1)]
                            nc.vector.tensor_mul(pt[:, 0:512], pt[:, 0:512], bt)
                            nc.vector.tensor_mul(pt[:, 512:1024],
                                                 pt[:, 512:1024], bt)
                        nc.tensor.matmul(
                            pva[:], v_sb[i][:, 0:65], pt[:, 0:512],
                            start=(n == 0), stop=(n == len(lv) - 1),
                            skip_group_check=True)
                        nc.tensor.matmul(
                            pvb[:], v_sb[i][:, 65:130], pt[:, 512:1024],
                            start=(n == 0), stop=(n == len(lv) - 1),
                            skip_group_check=True)
                    # copy PV results out of PSUM promptly (DVE) so the pv
                    # banks free up for the next pair regardless of gpsimd
                    au = at_pool.tile([65, 1024], BF16, tag="au")
                    nc.vector.tensor_copy(au[:, 0:512], pva[:])
                    nc.vector.tensor_copy(au[:, 512:1024], pvb[:])
                    # normalization: 1/denominator, broadcast, multiply.
                    # Keep everything bf16 so the DVE muls hit the 2x mode.
                    rsa = rc_pool.tile([1, 512], F32, tag="rsa")
                    rsb = rc_pool.tile([1, 512], F32, tag="rsb")
                    nc.vector.tensor_copy(rsa[:], au[64:65, 0:512])
                    nc.vector.tensor_copy(rsb[:], au[64:65, 512:1024])
                    rra = rc_pool.tile([1, 512], F32, tag="rra")
                    rrb = rc_pool.tile([1, 512], F32, tag="rrb")
                    nc.vector.reciprocal_approx_fast(out=rra[:], in_=rsa[:])
                    nc.vector.reciprocal_approx_fast(out=rrb[:], in_=rsb[:])
                    rh = rc_pool.tile([1, 1024], BF16, tag="rh")
                    nc.vector.tensor_copy(rh[:, 0:512], rra[:])
                    nc.vector.tensor_copy(rh[:, 512:1024], rrb[:])
                    rb = rc_pool.tile([64, 1024], BF16, tag="rb")
                    nc.gpsimd.partition_broadcast(rb[:, 0:512], rh[:, 0:512])
                    nc.gpsimd.partition_broadcast(rb[:, 512:1024],
                                                  rh[:, 512:1024])
                    ata = at_pool.tile([64, 1024], BF16, tag="ata")
                    nc.vector.tensor_mul(ata[:, 0:512], au[0:64, 0:512],
                                         rb[:, 0:512])
                    nc.vector.tensor_mul(ata[:, 512:1024],
                                         au[0:64, 512:1024], rb[:, 512:1024])
                    g2, p01 = pr // 2, pr % 2
                    nc.sync.dma_start(
                        ag_in[j][g2][128 * p01:128 * p01 + 64, :],
                        ata[:, 0:512])
                    nc.sync.dma_start(
                        ag_in[j][g2][128 * p01 + 64:128 * (p01 + 1), :],
                        ata[:, 512:1024])
                    if p01 == 1:
                        nc.gpsimd.collective_compute(
                            "AllGather", mybir.AluOpType.bypass,
                            replica_groups=groups,
                            ins=[ag_in[j][g2][:]], outs=[ag_out[j][g2][:]])
                        pending_scatters.append((j, g2, cur_blk[0]))

                pso_map = {}

                def oproj_half(t, half):
                    # half 0: bias + gathered g2=1 blocks (kk 8..15);
                    # half 1: g2=0 blocks (kk 0..7), then writeback.
                    ensure_scatter(t // 4, 1 if half == 0 else 0)
                    if half == 0:
                        pso = o_psum.tile([128, 512], F32, tag="pso")
                        pso_map[t] = pso
                        nc.tensor.matmul(pso[:], ones_sb[0:1, :], bo_sb[:],
                                         start=True, stop=False,
                                         skip_group_check=True)
                        for kk in range(8, 16):
                            nc.tensor.matmul(
                                pso[:], ag_sb[kk][:, 128 * t:128 * (t + 1)],
                                wo_sb[:, 512 * kk:512 * (kk + 1)],
                                start=False, stop=False, skip_group_check=True)
                    else:
                        pso = pso_map.pop(t)
                        for kk in range(8):
                            nc.tensor.matmul(
                                pso[:], ag_sb[kk][:, 128 * t:128 * (t + 1)],
                                wo_sb[:, 512 * kk:512 * (kk + 1)],
                                start=False, stop=(kk == 7),
                                skip_group_check=True)
                        osb = o_out.tile([128, 512], F32, tag="osb")
                        nc.vector.tensor_copy(osb[:], pso[:])
                        nc.sync.dma_start(out_ext[128 * t:128 * (t + 1), :],
                                          osb[:])

                PR_ORDER = [2, 3, 0, 1]
                oq = []   # entries: [t, next_half, ready0, ready1]
                cur_blk = [0]

                def pop_ready(blk_, budget):
                    # drain up to `budget` half-units.  Entries may be taken
                    # out of FIFO order (a ready h0 can jump a cc-gated h1),
                    # but at most 2 tiles may hold an o_psum slot (bufs=2).
                    while budget > 0:
                        n_open = sum(1 for e in oq if e[1] == 1)
                        done = False
                        for e in oq:
                            rdy = e[2] if e[1] == 0 else e[3]
                            if blk_ is not None and (rdy is None or rdy > blk_):
                                continue
                            if e[1] == 0 and n_open >= 2:
                                continue
                            oproj_half(e[0], e[1])
                            if e[1] == 0:
                                e[1] = 1
                            else:
                                oq.remove(e)
                            budget -= 1
                            done = True
                            break
                        if not done:
                            break
                for j in range(NJ):
                    for pr in PR_ORDER:
                        flush_scatters(cur_blk[0])
                        attn_block(j, pr)
                        cur_blk[0] += 1
                        blk = cur_blk[0]
                        if pr == 3:   # g2=1 AllGather just issued
                            slack = 2 if j == NJ - 1 else 4
                            for t in range(4 * j, 4 * j + 4):
                                oq.append([t, 0, blk + slack, None])
                        if pr == 1:   # g2=0 AllGather issued (chunk done)
                            for e in oq:
                                if 4 * j <= e[0] < 4 * j + 4:
                                    e[3] = blk + 4
                        pop_ready(blk, 2)
                flush_scatters(None)
                pop_ready(None, len(oq) * 2 + 4)

    nc.compile()
    _graph_cache[key] = nc
    return nc


def _pack16(a):
    """[2048, X] -> [128, 16*X] with row-chunk k at cols [X*k, X*(k+1))."""
    x = a.shape[1]
    return np.ascontiguousarray(
        a.reshape(16, 128, x).transpose(1, 0, 2).reshape(128, 16 * x))


def kernel(query, kv, Wq, bq, Wkv, bkv, Wo, bo, attn_mask, key_padding_mask):
    global last_results
    query = np.asarray(query, np.float32)
    kv = np.asarray(kv, np.float32)
    Wq = np.asarray(Wq, np.float32)
    bq = np.asarray(bq, np.float32)
    Wkv = np.asarray(Wkv, np.float32)
    bkv = np.asarray(bkv, np.float32)
    Wo = np.asarray(Wo, np.float32)
    bo = np.asarray(bo, np.float32)
    attn_mask = np.asarray(attn_mask, np.float32)
    kpm = np.asarray(key_padding_mask)

    eff = [attn_mask + np.where(kpm[b], np.float32(-1e9), np.float32(0.0))[None, :]
           for b in range(B)]
    live, band_list = _classify_blocks(eff)
    live_key = tuple((j, tuple(lv)) for j, lv in sorted(live.items()))
    band_key = tuple(band_list)

    nc = _build_graph(live_key, band_key)

    # Host-side shard prep (bf16 for all TensorEngine operands)
    qTh = [np.ascontiguousarray(query[b].T.astype(NPBF16)) for b in range(B)]
    kvTh = [np.ascontiguousarray(kv[b].T.astype(NPBF16)) for b in range(B)]
    bandh = []
    with np.errstate(over="ignore", under="ignore"):
        for b in range(B):
            if band_list:
                bt = np.stack(
                    [np.exp(eff[b][j * LQC:(j + 1) * LQC,
                                   i * KT:(i + 1) * KT].T)
                     for (j, i) in band_list]).astype(NPBF16)
                bandh.append(np.ascontiguousarray(
                    bt.transpose(1, 0, 2).reshape(KT, len(band_list) * LQC)))
            else:
                bandh.append(np.zeros((KT, 512), NPBF16))
    ones_h = np.ones((128, 128), NPBF16)
    ident_h = np.eye(128, dtype=NPBF16)

    Wq_h = Wq.reshape(HID, NH, D)
    bq_h = bq.reshape(NH, D)
    Wo_h = Wo.reshape(NH, D, HID)

    in_maps = []
    for c in range(N_CORES):
        b, r = c // TPR, c % TPR
        heads_q = [8 * r + pr + 4 * e for pr in range(NPAIR) for e in range(2)]
        perm_glob = [8 * rp + (2 * g2 + p01) + 4 * e
                     for g2 in range(2) for rp in range(TPR)
                     for p01 in range(2) for e in range(2)]
        wq_c = _pack16(
            (Wq_h[:, heads_q, :].reshape(HID, 512) * SCALE).astype(NPBF16))
        bq_c = np.ascontiguousarray(
            (bq_h[heads_q].reshape(512) * SCALE).reshape(4, 128).T)
        wk_c = _pack16(Wkv[:, 128 * r:128 * (r + 1)].astype(NPBF16))
        bk_c = np.ascontiguousarray(bkv[128 * r:128 * (r + 1)])[:, None]
        wv_c = _pack16(Wkv[:, 512 + 128 * r:512 + 128 * (r + 1)].astype(NPBF16))
        bv_c = np.ascontiguousarray(
            bkv[512 + 128 * r:512 + 128 * (r + 1)])[:, None]
        wo_c = _pack16(
            Wo_h[perm_glob].reshape(HID, HID)[:, 512 * r:512 * (r + 1)]
            .astype(NPBF16))
        bo_c = np.ascontiguousarray(
            bo[512 * r:512 * (r + 1)].astype(NPBF16))[None, :]
        in_maps.append({
            "qT": qTh[b], "kvT": kvTh[b],
            "wq": wq_c, "bq": bq_c,
            "wk": wk_c, "bk": bk_c,
            "wv": wv_c, "bv": bv_c,
            "wo": wo_c, "bo": bo_c,
            "band": bandh[b], "ones": ones_h, "ident": ident_h,
        })

    last_results = run_bass_kernel_spmd(nc, in_maps, core_ids=list(range(N_CORES)))

    out = np.empty((B, L, HID), np.float32)
    for c in range(N_CORES):
        b, r = c // TPR, c % TPR
        out[b, :, 512 * r:512 * (r + 1)] = last_results.results[c]["out"]
    return out


# revision 45
# speedup vs baseline: 1.1400x; 1.0045x over previous
"""GQA attention (B=2, L=2048, HID=2048, 32 Q heads / 8 KV heads) on 8 TRN2 cores.

Sharding: data-parallel on batch (2) x tensor-parallel on heads (4).
Core c: batch b = c//4, TP rank r = c%4 owns q heads {8r..8r+7} (whole GQA
groups: kv heads 2r, 2r+1). Compute in bf16 on the TensorEngine (fp32 PSUM
accumulation), fp32 softmax statistics. Per-core pipeline:
  1. kT = Wk_c.T @ kv_b.T and vT = Wv_c.T @ kv_b.T streamed over 16 row
     chunks of kvT; vT transposed back to [pos, dim] tiles via TensorE.
  2. QT = (Wq_c * scale).T @ query_b.T -> [512, L] bf16 (pair-major rows).
  3. per head-pair (g0-head, g1-head) and q-chunk j: scores^T = kT.T @ QT
     (two row-packed K=64 matmuls), exp on ACT, multiplicative mask
     (host-exp'd band tiles) on DVE, PV^T with ones-column giving the
     softmax denominator in PSUM row 64. PV results are copied out of PSUM
     immediately on DVE (releasing banks); normalization (reciprocal +
     gpsimd partition broadcast + mul) happens off the critical path.
  4. AllGather attnT (bf16) per (q-chunk, pair-group) over the 4-rank TP
     group.  Collectives are the ONLY thing on the gpsimd queue besides
     broadcasts, so their serialization does not stall the attention pipe.
  5. out_c[:, 512r:+512] = attnT_full.T @ Wo_perm_c + bo_c, split into two
     8-matmul half-accumulations per 128-row tile, interleaved into the
     attention instruction stream with enough slack to cover AG latency.
Host assembles [2, 2048, 2048] from per-core [2048, 512] f32 slabs.

Mask handling is input-driven: the effective additive mask (attn_mask +
key-padding) is classified on host per (q-chunk, k-tile) block as
all-masked (skip), all-zero (no op), or band (exp(mask) shipped and
multiplied into exp(scores)).
"""

import numpy as np
import ml_dtypes
import concourse.bass as bass
import concourse.mybir as mybir
import concourse.tile as tile
from concourse import bacc
from concourse.bass_utils import run_bass_kernel_spmd

F32 = mybir.dt.float32
BF16 = mybir.dt.bfloat16
AF = mybir.ActivationFunctionType
NPBF16 = ml_dtypes.bfloat16

B, L, HID = 2, 2048, 2048
NH, D, NKV = 32, 64, 8
SCALE = 0.125
N_CORES = 8
TPR = 4          # TP ranks per batch group
NPAIR = 4        # head pairs per core (g0-head, g1-head)
LQC = 512        # Lq chunk for attention (PSUM-bank sized)
NJ = L // LQC    # 4
KT = 128         # k-position tile
NI = L // KT     # 16
NEG_THRESH = -1.0e8
NCHUNK = HID // 128  # 16

_graph_cache = {}
last_results = None  # BassKernelResults of the most recent run (for test harness)


def _classify_blocks(eff_masks):
    """eff_masks: list of B arrays [L, L] (q, k). Returns (live, band_list)
    where live[j] is the ascending list of k-tiles to compute for q-chunk j and
    band_list orders the (j, i) blocks that need explicit mask values."""
    live = {}
    band_list = []
    for j in range(NJ):
        lv = []
        for i in range(NI):
            subs = [m[j * LQC:(j + 1) * LQC, i * KT:(i + 1) * KT] for m in eff_masks]
            if all((s <= NEG_THRESH).all() for s in subs):
                continue  # fully masked in every batch: contributes exactly 0
            lv.append(i)
            if not all((s == 0.0).all() for s in subs):
                band_list.append((j, i))
        live[j] = lv
    return live, band_list


def _build_graph(live_key, band_key):
    key = (live_key, band_key)
    if key in _graph_cache:
        return _graph_cache[key]

    live = {j: list(lv) for j, lv in live_key}
    band_list = list(band_key)
    band_idx = {ji: n for n, ji in enumerate(band_list)}
    nband = max(1, len(band_list))

    nc = bacc.Bacc("TRN2", target_bir_lowering=False, debug=False,
                   num_devices=N_CORES)

    qT = nc.dram_tensor("qT", [HID, L], BF16, kind="ExternalInput")
    kvT = nc.dram_tensor("kvT", [HID, L], BF16, kind="ExternalInput")
    wq = nc.dram_tensor("wq", [128, 16 * 512], BF16, kind="ExternalInput")
    bq = nc.dram_tensor("bq", [128, 4], F32, kind="ExternalInput")
    wk = nc.dram_tensor("wk", [128, 2048], BF16, kind="ExternalInput")
    bk = nc.dram_tensor("bk", [128, 1], F32, kind="ExternalInput")
    wv = nc.dram_tensor("wv", [128, 2048], BF16, kind="ExternalInput")
    bv = nc.dram_tensor("bv", [128, 1], F32, kind="ExternalInput")
    wo = nc.dram_tensor("wo", [128, 16 * 512], BF16, kind="ExternalInput")
    bo = nc.dram_tensor("bo", [1, 512], BF16, kind="ExternalInput")
    band = nc.dram_tensor("band", [128, nband * 512], BF16, kind="ExternalInput")
    ones = nc.dram_tensor("ones", [128, 128], BF16, kind="ExternalInput")
    ident = nc.dram_tensor("ident", [128, 128], BF16, kind="ExternalInput")
    out_ext = nc.dram_tensor("out", [L, 512], F32, kind="ExternalOutput")

    ag_warm_in = nc.dram_tensor("ag_warm_in", [256, 512], BF16)
    ag_warm_out = nc.dram_tensor("ag_warm_out", [1024, 512], BF16)
    ag_in = [[nc.dram_tensor(f"ag_in{j}_{g}", [256, 512], BF16)
              for g in range(2)] for j in range(NJ)]
    ag_out = [[nc.dram_tensor(f"ag_out{j}_{g}", [1024, 512], BF16)
               for g in range(2)] for j in range(NJ)]
    groups = [[0, 1, 2, 3], [4, 5, 6, 7]]

    with tile.TileContext(nc) as tc:
        with tc.tile_pool(name="persist", bufs=1) as persist:
            ones_sb = persist.tile([128, 128], BF16, tag="ones")
            nc.sync.dma_start(ones_sb[:], ones[:])
            # weights needed first: wk/wv lead the scalar queue
            wk_sb = persist.tile([128, 2048], BF16, tag="wk")
            nc.scalar.dma_start(wk_sb[:], wk[:])
            wv_sb = persist.tile([128, 2048], BF16, tag="wv")
            nc.scalar.dma_start(wv_sb[:], wv[:])
            # warm up the collective path with the REAL transfer shape so the
            # first in-loop AllGather doesn't pay the plan-setup cost
            nc.gpsimd.dma_start(ag_warm_in[0:128, 0:128], ones[:])
            nc.gpsimd.dma_start(ag_warm_in[128:256, 0:128], ones[:])
            nc.gpsimd.collective_compute(
                "AllGather", mybir.AluOpType.bypass,
                replica_groups=groups,
                ins=[ag_warm_in[:]], outs=[ag_warm_out[:]])

            ident_sb = persist.tile([128, 128], BF16, tag="ident")
            nc.scalar.dma_start(ident_sb[:], ident[:])
            bq_sb = persist.tile([128, 4], F32, tag="bq")
            nc.scalar.dma_start(bq_sb[:], bq[:])
            bk_sb = persist.tile([128, 1], F32, tag="bk")
            nc.scalar.dma_start(bk_sb[:], bk[:])
            bv_sb = persist.tile([128, 1], F32, tag="bv")
            nc.scalar.dma_start(bv_sb[:], bv[:])
            bo_sb = persist.tile([1, 512], BF16, tag="bo")
            nc.scalar.dma_start(bo_sb[:], bo[:])
            wq_sb = persist.tile([128, 16 * 512], BF16, tag="wq")
            nc.scalar.dma_start(wq_sb[:], wq[:])
            wo_sb = persist.tile([128, 16 * 512], BF16, tag="wo")
            band_sb = persist.tile([128, nband * 512], BF16, tag="band")

            kT_sb = persist.tile([128, L], BF16, tag="kT")
            v_sb = [persist.tile([128, 130], BF16, tag=f"v{t}", name=f"v{t}")
                    for t in range(NI)]
            for t in range(NI):
                nc.vector.memset(v_sb[t][:, 64:65], 1.0)
                nc.vector.memset(v_sb[t][:, 129:130], 1.0)
            QT_sb = [persist.tile([128, L], BF16, tag=f"qt{m}", name=f"qt{m}")
                     for m in range(NPAIR)]
            # gathered attnT blocks, filled per (j, g2) as AllGathers complete
            ag_sb = [persist.tile([128, L], BF16, tag=f"ag{kk}", name=f"ag{kk}")
                     for kk in range(16)]

            # ---- KV projection: kT and vT streamed over 16 row chunks
            with tc.tile_pool(name="kv_era", bufs=1) as kv_era:
                vT_sb = kv_era.tile([128, L], BF16, tag="vT")
                with (
                    tc.tile_pool(name="kv_stream", bufs=6) as kv_stream,
                    tc.tile_pool(name="kv_psum", bufs=1, space="PSUM") as kv_psum,
                ):
                    psk = kv_psum.tile([128, L], F32, tag="psk")
                    psv = kv_psum.tile([128, L], F32, tag="psv")
                    for k in range(NCHUNK):
                        ch = kv_stream.tile([128, L], BF16, tag="kvch")
                        # the whole kvT stream rides the sync ring so no
                        # chunk ever queues behind the weight loads on the
                        # scalar ring; two half-row DMAs so matmuls start
                        # on the first half while the second is in flight
                        nc.sync.dma_start(ch[:, 0:1024],
                                          kvT[128 * k:128 * (k + 1), 0:1024])
                        nc.sync.dma_start(ch[:, 1024:2048],
                                          kvT[128 * k:128 * (k + 1),
                                              1024:2048])
                        for jj in range(4):
                            nc.tensor.matmul(
                                psk[:, 512 * jj:512 * (jj + 1)],
                                wk_sb[:, 128 * k:128 * (k + 1)],
                                ch[:, 512 * jj:512 * (jj + 1)],
                                start=(k == 0), stop=(k == NCHUNK - 1),
                                skip_group_check=True)
                        for jj in range(4):
                            nc.tensor.matmul(
                                psv[:, 512 * jj:512 * (jj + 1)],
                                wv_sb[:, 128 * k:128 * (k + 1)],
                                ch[:, 512 * jj:512 * (jj + 1)],
                                start=(k == 0), stop=(k == NCHUNK - 1),
                                skip_group_check=True)
                    for jj in range(4):
                        nc.scalar.activation(
                            kT_sb[:, 512 * jj:512 * (jj + 1)],
                            psk[:, 512 * jj:512 * (jj + 1)],
                            AF.Identity, bias=bk_sb[:])
                        nc.scalar.activation(
                            vT_sb[:, 512 * jj:512 * (jj + 1)],
                            psv[:, 512 * jj:512 * (jj + 1)],
                            AF.Identity, bias=bv_sb[:])

                # transpose vT -> v tiles [pos, dim] (TensorE, exact)
                with tc.tile_pool(name="tr_psum", bufs=2, space="PSUM") as trp:
                    for t in range(NI):
                        tp = trp.tile([128, 128], BF16, tag="tp")
                        nc.tensor.transpose(
                            tp[:], vT_sb[:, 128 * t:128 * (t + 1)], ident_sb[:])
                        nc.vector.tensor_copy(v_sb[t][:, 0:64], tp[:, 0:64])
                        nc.vector.tensor_copy(v_sb[t][:, 65:129], tp[:, 64:128])

            # ---- Q projection (two 1024-col halves, 8 PSUM banks each)
            with (
                tc.tile_pool(name="q_stream", bufs=5) as q_stream,
                tc.tile_pool(name="q_psum", bufs=1, space="PSUM") as q_psum,
            ):
                for jp in range(2):
                    psq = [q_psum.tile([128, 1024], F32, tag=f"psq{m}",
                                       name=f"psq{m}") for m in range(4)]
                    for k in range(NCHUNK):
                        ch = q_stream.tile([128, 1024], BF16, tag="qch")
                        eng = nc.sync if k % 2 == 0 else nc.scalar
                        eng.dma_start(
                            ch[:, 0:512], qT[128 * k:128 * (k + 1),
                                             1024 * jp:1024 * jp + 512])
                        eng.dma_start(
                            ch[:, 512:1024], qT[128 * k:128 * (k + 1),
                                                1024 * jp + 512:
                                                1024 * (jp + 1)])
                        for m in range(4):
                            for jj in range(2):
                                nc.tensor.matmul(
                                    psq[m][:, 512 * jj:512 * (jj + 1)],
                                    wq_sb[:, 512 * k + 128 * m:
                                          512 * k + 128 * (m + 1)],
                                    ch[:, 512 * jj:512 * (jj + 1)],
                                    start=(k == 0), stop=(k == NCHUNK - 1),
                                    skip_group_check=True)
                    for m in range(4):
                        # split the final writeback between ACT and DVE so
                        # the PSUM pool hands off to the attention pools
                        # ~2x sooner after the last matmul
                        if jp == 1 and m >= 2:
                            nc.vector.tensor_scalar_add(
                                QT_sb[m][:, 1024 * jp:1024 * (jp + 1)],
                                psq[m][:], bq_sb[:, m:m + 1])
                        else:
                            for jj in range(2):
                                j = 2 * jp + jj
                                nc.scalar.activation(
                                    QT_sb[m][:, 512 * j:512 * (j + 1)],
                                    psq[m][:, 512 * jj:512 * (jj + 1)],
                                    AF.Identity, bias=bq_sb[:, m:m + 1])



            # weights for the attention era: band first (needed at j0),
            # wo later (needed from the first oproj unit)
            nc.scalar.dma_start(band_sb[:], band[:])
            nc.scalar.dma_start(wo_sb[:], wo[:])

            # ---- Attention (j-outer, pairs in order [2,3,0,1]) with the
            # ---- output projection interleaved as half-accumulations
            with (
                tc.tile_pool(name="pt_pool", bufs=4) as pt_pool,
                tc.tile_pool(name="at_pool", bufs=3) as at_pool,
                tc.tile_pool(name="rc_pool", bufs=2) as rc_pool,
                tc.tile_pool(name="o_out", bufs=2) as o_out,
                tc.tile_pool(name="qk_psum", bufs=2, space="PSUM") as qk_psum,
                tc.tile_pool(name="pv_psum", bufs=1, space="PSUM") as pv_psum,
                tc.tile_pool(name="o_psum", bufs=2, space="PSUM") as o_psum,
            ):
                pending_scatters = []

                def _emit_scatter(j_, g2_):
                    for rp in range(TPR):
                        for q01 in range(2):
                            kk = 8 * g2_ + 2 * rp + q01
                            nc.sync.dma_start(
                                ag_sb[kk][:, 512 * j_:512 * (j_ + 1)],
                                ag_out[j_][g2_][
                                    256 * rp + 128 * q01:
                                    256 * rp + 128 * (q01 + 1), :])

                def flush_scatters(now_blk):
                    # emit scatter DMAs only once the AllGather has had ~2
                    # pair-blocks to complete, so they never head-of-line
                    # block the sync queue for the ag_in writes behind them
                    while pending_scatters and (
                            now_blk is None
                            or pending_scatters[0][2] + 2 <= now_blk):
                        j_, g2_, _ = pending_scatters.pop(0)
                        _emit_scatter(j_, g2_)

                def ensure_scatter(j_, g2_):
                    for e in list(pending_scatters):
                        if e[0] == j_ and e[1] == g2_:
                            pending_scatters.remove(e)
                            _emit_scatter(j_, g2_)

                def attn_block(j, pr):
                    lv = live[j]
                    pva = pv_psum.tile([65, 512], F32, tag="pva")
                    pvb = pv_psum.tile([65, 512], F32, tag="pvb")
                    for n, i in enumerate(lv):
                        ps = qk_psum.tile([128, 1024], F32, tag="qk")
                        nc.tensor.matmul(
                            ps[:, 0:512],
                            kT_sb[0:64, 128 * i:128 * (i + 1)],
                            QT_sb[pr][0:64, 512 * j:512 * (j + 1)],
                            start=True, stop=True, skip_group_check=True)
                        nc.tensor.matmul(
                            ps[:, 512:1024],
                            kT_sb[64:128, 128 * i:128 * (i + 1)],
                            QT_sb[pr][64:128, 512 * j:512 * (j + 1)],
                            start=True, stop=True, skip_group_check=True)
                        pt = pt_pool.tile([128, 1024], BF16, tag="pt")
                        nc.scalar.activation(pt[:], ps[:], AF.Exp)
                        if (j, i) in band_idx:
                            nb = band_idx[(j, i)]
                            bt = band_sb[:, 512 * nb:512 * (nb + 1)]
                            nc.vector.tensor_mul(pt[:, 0:512], pt[:, 0:512], bt)
                            nc.vector.tensor_mul(pt[:, 512:1024],
                                                 pt[:, 512:1024], bt)
                        nc.tensor.matmul(
                            pva[:], v_sb[i][:, 0:65], pt[:, 0:512],
                            start=(n == 0), stop=(n == len(lv) - 1),
                            skip_group_check=True)
                        nc.tensor.matmul(
                            pvb[:], v_sb[i][:, 65:130], pt[:, 512:1024],
                            start=(n == 0), stop=(n == len(lv) - 1),
                            skip_group_check=True)
                    # copy PV results out of PSUM promptly (DVE) so the pv
                    # banks free up for the next pair regardless of gpsimd
                    au = at_pool.tile([65, 1024], BF16, tag="au")
                    nc.vector.tensor_copy(au[:, 0:512], pva[:])
                    nc.vector.tensor_copy(au[:, 512:1024], pvb[:])
                    # normalization: 1/denominator, broadcast, multiply.
                    # Keep everything bf16 so the DVE muls hit the 2x mode.
                    rsa = rc_pool.tile([1, 512], F32, tag="rsa")
                    rsb = rc_pool.tile([1, 512], F32, tag="rsb")
                    nc.vector.tensor_copy(rsa[:], au[64:65, 0:512])
                    nc.vector.tensor_copy(rsb[:], au[64:65, 512:1024])
                    rra = rc_pool.tile([1, 512], F32, tag="rra")
                    rrb = rc_pool.tile([1, 512], F32, tag="rrb")
                    nc.vector.reciprocal_approx_fast(out=rra[:], in_=rsa[:])
                    nc.vector.reciprocal_approx_fast(out=rrb[:], in_=rsb[:])
                    rh = rc_pool.tile([1, 1024], BF16, tag="rh")
                    nc.vector.tensor_copy(rh[:, 0:512], rra[:])
                    nc.vector.tensor_copy(rh[:, 512:1024], rrb[:])
                    rb = rc_pool.tile([64, 1024], BF16, tag="rb")
                    nc.gpsimd.partition_broadcast(rb[:, 0:512], rh[:, 0:512])
                    nc.gpsimd.partition_broadcast(rb[:, 512:1024],
                                                  rh[:, 512:1024])
                    ata = at_pool.tile([64, 1024], BF16, tag="ata")
                    nc.vector.tensor_mul(ata[:, 0:512], au[0:64, 0:512],
                                         rb[:, 0:512])
                    nc.vector.tensor_mul(ata[:, 512:1024],
                                         au[0:64, 512:1024], rb[:, 512:1024])
                    g2, p01 = pr // 2, pr % 2
                    nc.sync.dma_start(
                        ag_in[j][g2][128 * p01:128 * p01 + 64, :],
                        ata[:, 0:512])
                    nc.sync.dma_start(
                        ag_in[j][g2][128 * p01 + 64:128 * (p01 + 1), :],
                        ata[:, 512:1024])
                    if p01 == 1:
                        nc.gpsimd.collective_compute(
                            "AllGather", mybir.AluOpType.bypass,
                            replica_groups=groups,
                            ins=[ag_in[j][g2][:]], outs=[ag_out[j][g2][:]])
                        pending_scatters.append((j, g2, cur_blk[0]))

                pso_map = {}

                def oproj_half(t, half):
                    # half 0: bias + gathered g2=1 blocks (kk 8..15);
                    # half 1: g2=0 blocks (kk 0..7), then writeback.
                    ensure_scatter(t // 4, 1 if half == 0 else 0)
                    if half == 0:
                        pso = o_psum.tile([128, 512], F32, tag="pso")
                        pso_map[t] = pso
                        nc.tensor.matmul(pso[:], ones_sb[0:1, :], bo_sb[:],
                                         start=True, stop=False,
                                         skip_group_check=True)
                        for kk in range(8, 16):
                            nc.tensor.matmul(
                                pso[:], ag_sb[kk][:, 128 * t:128 * (t + 1)],
                                wo_sb[:, 512 * kk:512 * (kk + 1)],
                                start=False, stop=False, skip_group_check=True)
                    else:
                        pso = pso_map.pop(t)
                        for kk in range(8):
                            nc.tensor.matmul(
                                pso[:], ag_sb[kk][:, 128 * t:128 * (t + 1)],
                                wo_sb[:, 512 * kk:512 * (kk + 1)],
                                start=False, stop=(kk == 7),
                                skip_group_check=True)
                        osb = o_out.tile([128, 512], F32, tag="osb")
                        nc.vector.tensor_copy(osb[:], pso[:])
                        nc.sync.dma_start(out_ext[128 * t:128 * (t + 1), :],
                                          osb[:])

                PR_ORDER = [2, 3, 0, 1]
                oq = []   # entries: [t, next_half, ready0, ready1]
                cur_blk = [0]

                def pop_ready(blk_, budget):
                    # drain up to `budget` half-units.  Entries may be taken
                    # out of FIFO order (a ready h0 can jump a cc-gated h1),
                    # but at most 2 tiles may hold an o_psum slot (bufs=2).
                    while budget > 0:
                        n_open = sum(1 for e in oq if e[1] == 1)
                        done = False
                        for e in oq:
                            rdy = e[2] if e[1] == 0 else e[3]
                            if blk_ is not None and (rdy is None or rdy > blk_):
                                continue
                            if e[1] == 0 and n_open >= 2:
                                continue
                            oproj_half(e[0], e[1])
                            if e[1] == 0:
                                e[1] = 1
                            else:
                                oq.remove(e)
                            budget -= 1
                            done = True
                            break
                        if not done:
                            break
                for j in range(NJ):
                    for pr in PR_ORDER:
                        flush_scatters(cur_blk[0])
                        attn_block(j, pr)
                        cur_blk[0] += 1
                        blk = cur_blk[0]
                        if pr == 3:   # g2=1 AllGather just issued
                            slack = 2 if j == NJ - 1 else 4
                            for t in range(4 * j, 4 * j + 4):
                                oq.append([t, 0, blk + slack, None])
                        if pr == 1:   # g2=0 AllGather issued (chunk done)
                            for e in oq:
                                if 4 * j <= e[0] < 4 * j + 4:
                                    e[3] = blk + 4
                        pop_ready(blk, 2)
                flush_scatters(None)
                pop_ready(None, len(oq) * 2 + 4)

    nc.compile()
    _graph_cache[key] = nc
    return nc


def _pack16(a):
    """[2048, X] -> [128, 16*X] with row-chunk k at cols [X*k, X*(k+1))."""
    x = a.shape[1]
    return np.ascontiguousarray(
        a.reshape(16, 128, x).transpose(1, 0, 2).reshape(128, 16 * x))


def kernel(query, kv, Wq, bq, Wkv, bkv, Wo, bo, attn_mask, key_padding_mask):
    global last_results
    query = np.asarray(query, np.float32)
    kv = np.asarray(kv, np.float32)
    Wq = np.asarray(Wq, np.float32)
    bq = np.asarray(bq, np.float32)
    Wkv = np.asarray(Wkv, np.float32)
    bkv = np.asarray(bkv, np.float32)
    Wo = np.asarray(Wo, np.float32)
    bo = np.asarray(bo, np.float32)
    attn_mask = np.asarray(attn_mask, np.float32)
    kpm = np.asarray(key_padding_mask)

    eff = [attn_mask + np.where(kpm[b], np.float32(-1e9), np.float32(0.0))[None, :]
           for b in range(B)]
    live, band_list = _classify_blocks(eff)
    live_key = tuple((j, tuple(lv)) for j, lv in sorted(live.items()))
    band_key = tuple(band_list)

    nc = _build_graph(live_key, band_key)

    # Host-side shard prep (bf16 for all TensorEngine operands)
    qTh = [np.ascontiguousarray(query[b].T.astype(NPBF16)) for b in range(B)]
    kvTh = [np.ascontiguousarray(kv[b].T.astype(NPBF16)) for b in range(B)]
    bandh = []
    with np.errstate(over="ignore", under="ignore"):
        for b in range(B):
            if band_list:
                bt = np.stack(
                    [np.exp(eff[b][j * LQC:(j + 1) * LQC,
                                   i * KT:(i + 1) * KT].T)
                     for (j, i) in band_list]).astype(NPBF16)
                bandh.append(np.ascontiguousarray(
                    bt.transpose(1, 0, 2).reshape(KT, len(band_list) * LQC)))
            else:
                bandh.append(np.zeros((KT, 512), NPBF16))
    ones_h = np.ones((128, 128), NPBF16)
    ident_h = np.eye(128, dtype=NPBF16)

    Wq_h = Wq.reshape(HID, NH, D)
    bq_h = bq.reshape(NH, D)
    Wo_h = Wo.reshape(NH, D, HID)

    in_maps = []
    for c in range(N_CORES):
        b, r = c // TPR, c % TPR
        heads_q = [8 * r + pr + 4 * e for pr in range(NPAIR) for e in range(2)]
        perm_glob = [8 * rp + (2 * g2 + p01) + 4 * e
                     for g2 in range(2) for rp in range(TPR)
                     for p01 in range(2) for e in range(2)]
        wq_c = _pack16(
            (Wq_h[:, heads_q, :].reshape(HID, 512) * SCALE).astype(NPBF16))
        bq_c = np.ascontiguousarray(
            (bq_h[heads_q].reshape(512) * SCALE).reshape(4, 128).T)
        wk_c = _pack16(Wkv[:, 128 * r:128 * (r + 1)].astype(NPBF16))
        bk_c = np.ascontiguousarray(bkv[128 * r:128 * (r + 1)])[:, None]
        wv_c = _pack16(Wkv[:, 512 + 128 * r:512 + 128 * (r + 1)].astype(NPBF16))
        bv_c = np.ascontiguousarray(
            bkv[512 + 128 * r:512 + 128 * (r + 1)])[:, None]
        wo_c = _pack16(
            Wo_h[perm_glob].reshape(HID, HID)[:, 512 * r:512 * (r + 1)]
            .astype(NPBF16))
        bo_c = np.ascontiguousarray(
            bo[512 * r:512 * (r + 1)].astype(NPBF16))[None, :]
        in_maps.append({
            "qT": qTh[b], "kvT": kvTh[b],
            "wq": wq_c, "bq": bq_c,
            "wk": wk_c, "bk": bk_c,
            "wv": wv_c, "bv": bv_c,
            "wo": wo_c, "bo": bo_c,
            "band": bandh[b], "ones": ones_h, "ident": ident_h,
        })

    last_results = run_bass_kernel_spmd(nc, in_maps, core_ids=list(range(N_CORES)))

    out = np.empty((B, L, HID), np.float32)
    for c in range(N_CORES):
        b, r = c // TPR, c % TPR
        out[b, :, 512 * r:512 * (r + 1)] = last_results.results[c]["out"]
    return out
